# revision 8
# baseline (speedup 1.0000x reference)
"""Trainium2 Bass kernel for an 8-layer weight-shared decoder stack.

Model (see problem reference): h = emb[x]; 8x identical decoder layers
(LN -> single-head attn tiled 16x -> proj -> LN -> 4x FFN); fc to vocab.

Distribution over 8 NeuronCores:
  - tokens sharded 8-way (cores 0-3 <- batch 0, cores 4-7 <- batch 1;
    512 tokens per core); per-layer AllGather of K/V within each 4-core
    batch group;
  - final hidden states AllGathered across all 8 cores; fc vocab-sharded
    (4000 columns per core); host concatenates the vocab shards.

Numerics: fp32r matmuls (full PE rate at N>=256), fp32 residual stream.
Algebraic folds: tile(head,16) @ Wd == head @ Wd_sum; LN affine (g, beta)
folded into the following weight matrices; softmax denominator applied
to the AV product instead of the probabilities (linearity).
Activations are stored transposed (embedding on partitions) so no
activation transposes are needed anywhere; attention scores are computed
directly in [key, query] layout and the softmax reductions over keys run
on the PE via ones-vector matmuls.
"""
import numpy as np
from contextlib import ExitStack

import concourse.bass as bass
import concourse.tile as tile
from concourse import bacc, mybir
from concourse.bass_utils import run_bass_kernel_spmd
from concourse.masks import make_identity

dt = mybir.dt
AF = mybir.ActivationFunctionType
ALU = mybir.AluOpType

# model dims (hardcoded per the problem spec)
VOCAB, EMB, SEQ, STACK, N_HEADS, ATTN, BATCH = 32000, 1024, 2048, 8, 16, 64, 2
N_CORES = 8
T = (BATCH * SEQ) // N_CORES          # 512 tokens per core
GRP = 4                               # cores per batch group
GROUPS = [[0, 1, 2, 3], [4, 5, 6, 7]]
EC = EMB // 128                       # 8 emb chunks
KC = SEQ // 128                       # 16 key chunks (per batch)
HC = 4 * EMB // 128                   # 32 ffn hidden chunks
TC = T // 128                         # 4 local token chunks
VSH = VOCAB // N_CORES                # 4000 vocab per core
VCC = 8                               # vocab col chunks per core
VCW = VSH // VCC                      # 500 cols per chunk
GTC = (BATCH * SEQ) // 128            # 32 global token chunks
F32, F32R, I32 = dt.float32, dt.float32r, dt.int32


def build_nc():
    nc = bacc.Bacc("TRN2", target_bir_lowering=False, debug=False,
                   enable_asserts=True, num_devices=N_CORES)

    # ---- I/O ----
    emb = nc.dram_tensor("emb", [VOCAB, EMB], F32, kind="ExternalInput").ap()
    xi = nc.dram_tensor("xi", [T, 1], I32, kind="ExternalInput").ap()
    wq = nc.dram_tensor("wq", [EMB, ATTN], F32R, kind="ExternalInput").ap()
    wk = nc.dram_tensor("wk", [EMB, ATTN], F32R, kind="ExternalInput").ap()
    wv = nc.dram_tensor("wv", [EMB, ATTN], F32R, kind="ExternalInput").ap()
    bqkv = nc.dram_tensor("bqkv", [ATTN, 3], F32, kind="ExternalInput").ap()
    wd = nc.dram_tensor("wd", [ATTN, EMB], F32R, kind="ExternalInput").ap()  # Wd_sum
    bd = nc.dram_tensor("bd", [1, EMB], F32R, kind="ExternalInput").ap()
    w1 = nc.dram_tensor("w1", [EMB, 4 * EMB], F32R, kind="ExternalInput").ap()
    c1 = nc.dram_tensor("c1", [128, HC], F32, kind="ExternalInput").ap()
    w2 = nc.dram_tensor("w2", [4 * EMB, EMB], F32R, kind="ExternalInput").ap()
    c2 = nc.dram_tensor("c2", [1, EMB], F32R, kind="ExternalInput").ap()
    wfc = nc.dram_tensor("wfc", [EMB, VSH], F32R, kind="ExternalInput").ap()
    bfc = nc.dram_tensor("bfc", [1, VSH], F32R, kind="ExternalInput").ap()
    out = nc.dram_tensor("out", [BATCH * SEQ, VSH], F32, kind="ExternalOutput").ap()

    with tile.TileContext(nc) as tc, ExitStack() as ctx:
        dram = ctx.enter_context(tc.tile_pool(name="dram", bufs=1, space="DRAM"))
        consts = ctx.enter_context(tc.tile_pool(name="consts", bufs=1))
        ps_mm = ctx.enter_context(tc.tile_pool(name="ps_mm", bufs=2, space="PSUM"))
        ps_st = ctx.enter_context(tc.tile_pool(name="ps_st", bufs=2, space="PSUM"))
        ps_v64 = ctx.enter_context(tc.tile_pool(name="ps_v64", bufs=2, space="PSUM"))
        ps_b = ctx.enter_context(tc.tile_pool(name="ps_b", bufs=1, space="PSUM"))

        # ---- constants / weights resident in SBUF ----
        ident = consts.tile([128, 128], F32, tag="ident")
        make_identity(nc, ident[:])
        ones_f = consts.tile([128, 1], F32, tag="ones_f")
        nc.vector.memset(ones_f[:], 1.0)
        onesc = consts.tile([128, 1], F32R, tag="onesc")     # [128,1] ones column
        nc.vector.tensor_copy(onesc[:], ones_f[:])
        ones_rowf = consts.tile([1, T], F32, tag="ones_rowf")
        nc.vector.memset(ones_rowf[:], 1.0)
        onesr = consts.tile([1, T], F32R, tag="onesr")       # [1,512] ones row
        nc.vector.tensor_copy(onesr[:], ones_rowf[:])
        eps_t = consts.tile([1, 1], F32, tag="eps")
        nc.vector.memset(eps_t[:], 1e-5)

        wq_t = consts.tile([128, EC * ATTN], F32R, tag="wq")
        wk_t = consts.tile([128, EC * ATTN], F32R, tag="wk")
        wv_t = consts.tile([128, EC * ATTN], F32R, tag="wv")
        for w_t, w_d in ((wq_t, wq), (wk_t, wk), (wv_t, wv)):
            nc.sync.dma_start(
                w_t.rearrange("p (ec a) -> p ec a", ec=EC),
                w_d.rearrange("(ec p) a -> p ec a", p=128))
        bqkv_t = consts.tile([ATTN, 3], F32, tag="bqkv")
        nc.sync.dma_start(bqkv_t[:], bqkv)
        wd_t = consts.tile([ATTN, EMB], F32R, tag="wd")
        nc.sync.dma_start(wd_t[:], wd)
        bd_t = consts.tile([1, EMB], F32R, tag="bd")
        nc.sync.dma_start(bd_t[:], bd)
        c1_t = consts.tile([128, HC], F32, tag="c1")
        nc.sync.dma_start(c1_t[:], c1)
        c2_t = consts.tile([1, EMB], F32R, tag="c2")
        nc.sync.dma_start(c2_t[:], c2)

        # final-hidden gather buffers (phase boundary)
        hg_in = dram.tile([EMB, T], F32R, tag="hg_in")
        hg = dram.tile([N_CORES, EMB, T], F32R, tag="hg")

        # ================= phase 1: embed + decoder stack =================
        with ExitStack() as lctx:
            hp = lctx.enter_context(tc.tile_pool(name="hpool", bufs=1))
            lay = lctx.enter_context(tc.tile_pool(name="lay", bufs=2))
            scr = lctx.enter_context(tc.tile_pool(name="scratch", bufs=2))
            abp = lctx.enter_context(tc.tile_pool(name="abp", bufs=1))
            a1p = lctx.enter_context(tc.tile_pool(name="a1p", bufs=1))
            w1p = lctx.enter_context(tc.tile_pool(name="w1p", bufs=3))
            w2p = lctx.enter_context(tc.tile_pool(name="w2p", bufs=2))
            etp = lctx.enter_context(tc.tile_pool(name="etp", bufs=3))
            kvp = lctx.enter_context(tc.tile_pool(name="kvp", bufs=1))
            rows = lctx.enter_context(tc.tile_pool(name="rows", bufs=4))
            rows2 = lctx.enter_context(tc.tile_pool(name="rows2", bufs=2))
            up = lctx.enter_context(tc.tile_pool(name="up", bufs=3))
            embp = lctx.enter_context(tc.tile_pool(name="embp", bufs=2))

            # residual hT: [emb-part, token-free], chunk ec at cols [ec*T,(ec+1)*T)
            h_t = hp.tile([128, EC * T], F32, tag="h")

            def hcol(ec):
                return h_t[:, ec * T:(ec + 1) * T]

            # ---- embedding gather + transpose ----
            with nc.named_scope("embed"):
                for tk in range(TC):
                    idx_t = embp.tile([128, 1], I32, tag="idx")
                    nc.sync.dma_start(idx_t[:], xi[tk * 128:(tk + 1) * 128, :])
                    gat = embp.tile([128, EMB], F32, tag="gat")
                    nc.gpsimd.indirect_dma_start(
                        out=gat[:], out_offset=None, in_=emb,
                        in_offset=bass.IndirectOffsetOnAxis(ap=idx_t[:, :1], axis=0))
                    for ec in range(EC):
                        tr_ps = ps_mm.tile([128, 128], F32, tag="mm")
                        nc.tensor.transpose(
                            tr_ps[:], gat[:, ec * 128:(ec + 1) * 128], ident[:])
                        nc.vector.tensor_copy(
                            h_t[:, ec * T + tk * 128: ec * T + (tk + 1) * 128],
                            tr_ps[:])

            def layernorm(z_t):
                """z = (h - mu(h)) / sqrt(var(h)+eps), fp32r into z_t."""
                hr = scr.tile([128, EC * T], F32R, tag="scr4")
                nc.vector.tensor_copy(hr[:], h_t[:])
                hsq = scr.tile([128, EC * T], F32R, tag="scr4")
                nc.vector.tensor_tensor(hsq[:], h_t[:], h_t[:], op=ALU.mult)
                sum_ps = ps_st.tile([1, T], F32, tag="stat")
                sq_ps = ps_st.tile([1, T], F32, tag="stat")
                for ec in range(EC):
                    nc.tensor.matmul(sum_ps[:], onesc[:], hr[:, ec * T:(ec + 1) * T],
                                     start=(ec == 0), stop=(ec == EC - 1))
                for ec in range(EC):
                    nc.tensor.matmul(sq_ps[:], onesc[:], hsq[:, ec * T:(ec + 1) * T],
                                     start=(ec == 0), stop=(ec == EC - 1))
                nmu = rows.tile([1, T], F32, tag="r1")
                nc.vector.tensor_scalar(nmu[:], sum_ps[:], -1.0 / EMB, None,
                                        op0=ALU.mult)
                var = rows.tile([1, T], F32, tag="r1")
                nc.vector.tensor_scalar(var[:], sq_ps[:], 1.0 / EMB, None,
                                        op0=ALU.mult)
                musq = rows.tile([1, T], F32, tag="r1")
                nc.vector.tensor_tensor(musq[:], nmu[:], nmu[:], op=ALU.mult)
                nc.vector.tensor_tensor(var[:], var[:], musq[:], op=ALU.subtract)
                sd = rows.tile([1, T], F32, tag="r1")
                nc.scalar.activation(sd[:], var[:], AF.Sqrt, bias=eps_t[:])
                istd = rows.tile([1, T], F32, tag="r1")
                nc.vector.reciprocal(istd[:], sd[:])
                ab_row = rows2.tile([1, 2 * T], F32R, tag="r2")
                nc.vector.tensor_copy(ab_row[:, :T], istd[:])
                nc.vector.tensor_tensor(ab_row[:, T:], nmu[:], istd[:], op=ALU.mult)
                ab_ps = ps_b.tile([128, 2 * T], F32, tag="bcast")
                nc.tensor.matmul(ab_ps[:, :T], onesr[:, :128], ab_row[:, :T],
                                 start=True, stop=True)
                nc.tensor.matmul(ab_ps[:, T:], onesr[:, :128], ab_row[:, T:],
                                 start=True, stop=True)
                ab_sb = abp.tile([128, 2 * T], F32, tag="ab")
                nc.vector.tensor_copy(ab_sb[:], ab_ps[:])
                for ec in range(EC):
                    u = up.tile([128, T], F32, tag="u")
                    nc.vector.tensor_tensor(u[:], hcol(ec), ab_sb[:, :T],
                                            op=ALU.mult)
                    nc.vector.tensor_tensor(z_t[:, ec * T:(ec + 1) * T], u[:],
                                            ab_sb[:, T:], op=ALU.add)

            for layer in range(STACK):
                with nc.named_scope(f"L{layer}"):
                    # ---- LN1 + QKV ----
                    z_t = scr.tile([128, EC * T], F32R, tag="scr4")
                    layernorm(z_t)
                    qkv_sb = []
                    for qi, w_t in enumerate((wq_t, wk_t, wv_t)):
                        p = ps_v64.tile([ATTN, T], F32, tag="vec64")
                        for ec in range(EC):
                            nc.tensor.matmul(
                                p[:], w_t[:, ec * ATTN:(ec + 1) * ATTN],
                                z_t[:, ec * T:(ec + 1) * T],
                                start=(ec == 0), stop=(ec == EC - 1))
                        s = lay.tile([ATTN, T], F32R, tag=f"qkv{qi}")
                        nc.scalar.activation(s[:], p[:], AF.Identity,
                                             bias=bqkv_t[:, qi:qi + 1])
                        qkv_sb.append(s)
                    qT, kT_loc, vT_loc = qkv_sb

                    # local v -> token-major [128, TC*64]
                    v_loc = lay.tile([128, TC * ATTN], F32R, tag="vloc")
                    for tk in range(TC):
                        tp = ps_v64.tile([128, 128], F32, tag="vec64")
                        nc.tensor.transpose(
                            tp[:128, :ATTN],
                            vT_loc[:, tk * 128:(tk + 1) * 128].bitcast(F32),
                            ident[:ATTN, :ATTN])
                        nc.vector.tensor_copy(
                            v_loc[:, tk * ATTN:(tk + 1) * ATTN], tp[:128, :ATTN])

                    # ---- gather K/V across the 4-core batch group ----
                    kv_loc = dram.tile([2 * ATTN * T], F32R, tag="kv_loc")
                    nc.sync.dma_start(
                        kv_loc[0:ATTN * T].rearrange("(a t) -> a t", a=ATTN),
                        kT_loc[:])
                    nc.sync.dma_start(
                        kv_loc[ATTN * T:].rearrange("(p c) -> p c", p=128),
                        v_loc[:])
                    kv_g = dram.tile([GRP, 2 * ATTN * T], F32R, tag="kv_g")
                    nc.gpsimd.collective_compute(
                        "AllGather", ALU.bypass, replica_groups=GROUPS,
                        ins=[kv_loc.opt()], outs=[kv_g.opt()])
                    kT = kvp.tile([ATTN, SEQ], F32R, tag="kT")
                    vtm = kvp.tile([128, KC * ATTN], F32R, tag="vtm")
                    for r in range(GRP):
                        nc.sync.dma_start(
                            kT[:, r * T:(r + 1) * T],
                            kv_g[r, 0:ATTN * T].rearrange("(a t) -> a t", a=ATTN))
                        nc.sync.dma_start(
                            vtm[:, r * TC * ATTN:(r + 1) * TC * ATTN]
                            .rearrange("p (c a) -> p c a", c=TC),
                            kv_g[r, ATTN * T:]
                            .rearrange("(p c a) -> p c a", p=128, c=TC))

                    # ---- attention ----
                    # e = exp(scoresT); AV and denominator accumulate per chunk;
                    # the 1/denominator is applied to the AV product (linearity)
                    den_ps = ps_st.tile([1, T], F32, tag="stat")
                    head_ps = ps_v64.tile([ATTN, T], F32, tag="vec64")
                    for kc in range(KC):
                        s_ps = ps_mm.tile([128, T], F32, tag="mm")
                        nc.tensor.matmul(s_ps[:], kT[:, kc * 128:(kc + 1) * 128],
                                         qT[:], start=True, stop=True)
                        e_kc = etp.tile([128, T], F32R, tag="eT")
                        nc.scalar.activation(e_kc[:], s_ps[:], AF.Exp,
                                             scale=float(ATTN) ** -0.5)
                        nc.tensor.matmul(den_ps[:], onesc[:], e_kc[:],
                                         start=(kc == 0), stop=(kc == KC - 1))
                        nc.tensor.matmul(head_ps[:],
                                         vtm[:, kc * ATTN:(kc + 1) * ATTN],
                                         e_kc[:],
                                         start=(kc == 0), stop=(kc == KC - 1))
                    recip = rows.tile([1, T], F32, tag="r1")
                    nc.vector.reciprocal(recip[:], den_ps[:])
                    rrow = rows.tile([1, T], F32R, tag="r1")
                    nc.vector.tensor_copy(rrow[:], recip[:])
                    rb_ps = ps_b.tile([128, 2 * T], F32, tag="bcast")
                    nc.tensor.matmul(rb_ps[:ATTN, :T], onesr[:, :ATTN], rrow[:],
                                     start=True, stop=True)
                    rb_sb = abp.tile([ATTN, T], F32, tag="rb")
                    nc.vector.tensor_copy(rb_sb[:], rb_ps[:ATTN, :T])
                    headT = lay.tile([ATTN, T], F32R, tag="headT")
                    nc.vector.tensor_tensor(headT[:], head_ps[:], rb_sb[:],
                                            op=ALU.mult)

                    # ---- proj + residual ----
                    for ec in range(EC):
                        p_ps = ps_mm.tile([128, T], F32, tag="mm")
                        nc.tensor.matmul(p_ps[:], wd_t[:, ec * 128:(ec + 1) * 128],
                                         headT[:], start=True, stop=False)
                        nc.tensor.matmul(p_ps[:], bd_t[:, ec * 128:(ec + 1) * 128],
                                         onesr[:], start=False, stop=True)
                        nc.vector.tensor_tensor(hcol(ec), hcol(ec), p_ps[:],
                                                op=ALU.add)

                    # ---- LN2 + FFN (two half passes over hidden chunks) ----
                    z2_t = scr.tile([128, EC * T], F32R, tag="scr4")
                    layernorm(z2_t)
                    for half in range(2):
                        a1 = a1p.tile([128, (HC // 2) * T], F32R, tag="a1")
                        for j in range(HC // 2):
                            hc = half * (HC // 2) + j
                            w1_t = w1p.tile([128, EC * 128], F32R, tag="w1")
                            nc.sync.dma_start(
                                w1_t.rearrange("p (ec m) -> p ec m", ec=EC),
                                w1[:, hc * 128:(hc + 1) * 128]
                                .rearrange("(ec p) m -> p ec m", p=128))
                            f_ps = ps_mm.tile([128, T], F32, tag="mm")
                            for ec in range(EC):
                                nc.tensor.matmul(
                                    f_ps[:], w1_t[:, ec * 128:(ec + 1) * 128],
                                    z2_t[:, ec * T:(ec + 1) * T],
                                    start=(ec == 0), stop=(ec == EC - 1))
                            nc.scalar.activation(a1[:, j * T:(j + 1) * T], f_ps[:],
                                                 AF.Relu, bias=c1_t[:, hc:hc + 1])
                        for ec in range(EC):
                            w2_t = w2p.tile([128, (HC // 2) * 128], F32R, tag="w2")
                            nc.sync.dma_start(
                                w2_t.rearrange("p (j m) -> p j m", j=HC // 2),
                                w2[half * 2048:(half + 1) * 2048,
                                   ec * 128:(ec + 1) * 128]
                                .rearrange("(j p) m -> p j m", p=128))
                            g_ps = ps_mm.tile([128, T], F32, tag="mm")
                            for j in range(HC // 2):
                                nc.tensor.matmul(
                                    g_ps[:], w2_t[:, j * 128:(j + 1) * 128],
                                    a1[:, j * T:(j + 1) * T],
                                    start=(j == 0),
                                    stop=(j == HC // 2 - 1 and half == 1))
                            if half == 1:
                                nc.tensor.matmul(
                                    g_ps[:], c2_t[:, ec * 128:(ec + 1) * 128],
                                    onesr[:], start=False, stop=True)
                            nc.vector.tensor_tensor(hcol(ec), hcol(ec), g_ps[:],
                                                    op=ALU.add)

            # ---- round h and stage it for the all-core gather ----
            with nc.named_scope("hgather"):
                hfin = scr.tile([128, EC * T], F32R, tag="scr4")
                nc.vector.tensor_copy(hfin[:], h_t[:])
                for ec in range(EC):
                    nc.sync.dma_start(hg_in[ec * 128:(ec + 1) * 128, :],
                                      hfin[:, ec * T:(ec + 1) * T])

        nc.gpsimd.collective_compute(
            "AllGather", ALU.bypass,
            replica_groups=[list(range(N_CORES))],
            ins=[hg_in.opt()], outs=[hg.opt()])

        # ================= phase 2: fc over the vocab shard =================
        # two token-half passes so the resident hT stays at 64KB/partition
        with nc.named_scope("fc"):
            with tc.tile_pool(name="fcp", bufs=1) as fcp, \
                 tc.tile_pool(name="wfcp", bufs=2) as wfcp, \
                 tc.tile_pool(name="outp", bufs=4) as outp, \
                 tc.tile_pool(name="bfcp", bufs=1) as bfcp:
                bfc_t = bfcp.tile([1, VSH], F32R, tag="bfc")
                nc.sync.dma_start(bfc_t[:], bfc)
                HGT = GTC // 2  # global token chunks per half
                for th in range(2):
                    hT_half = fcp.tile([128, EC * HGT * 128], F32R, tag="hT")
                    for ec in range(EC):
                        for r in range(N_CORES // 2):
                            rr = th * (N_CORES // 2) + r
                            nc.sync.dma_start(
                                hT_half[:, ec * HGT * 128 + r * T:
                                        ec * HGT * 128 + (r + 1) * T],
                                hg[rr, ec * 128:(ec + 1) * 128, :])
                    for vc in range(VCC):
                        wfc_t = wfcp.tile([128, EC * VCW], F32R, tag="wfc")
                        nc.sync.dma_start(
                            wfc_t.rearrange("p (ec n) -> p ec n", ec=EC),
                            wfc[:, vc * VCW:(vc + 1) * VCW]
                            .rearrange("(ec p) n -> p ec n", p=128))
                        for tcg in range(HGT):
                            o_ps = ps_mm.tile([128, VCW], F32, tag="mm")
                            for ec in range(EC):
                                nc.tensor.matmul(
                                    o_ps[:],
                                    hT_half[:, ec * HGT * 128 + tcg * 128:
                                            ec * HGT * 128 + (tcg + 1) * 128],
                                    wfc_t[:, ec * VCW:(ec + 1) * VCW],
                                    start=(ec == 0), stop=False)
                            nc.tensor.matmul(o_ps[:], onesr[:, :128],
                                             bfc_t[:, vc * VCW:(vc + 1) * VCW],
                                             start=False, stop=True)
                            o_sb = outp.tile([128, VCW], F32, tag="osb")
                            nc.vector.tensor_copy(o_sb[:], o_ps[:])
                            gt = th * HGT + tcg
                            nc.sync.dma_start(
                                out[gt * 128:(gt + 1) * 128,
                                    vc * VCW:(vc + 1) * VCW], o_sb[:])

    nc.compile()
    return nc


_NC_CACHE = None


def _get_nc():
    global _NC_CACHE
    if _NC_CACHE is None:
        _NC_CACHE = build_nc()
    return _NC_CACHE


def prepare_in_maps(inputs):
    f32 = np.float32
    x = np.asarray(inputs["x"]).reshape(-1).astype(np.int32)
    emb = np.ascontiguousarray(np.asarray(inputs["emb"], f32))
    g1 = np.asarray(inputs["g1"], f32)
    beta1 = np.asarray(inputs["beta1"], f32)
    g2 = np.asarray(inputs["g2"], f32)
    beta2 = np.asarray(inputs["beta2"], f32)
    Wq = np.asarray(inputs["Wq"], f32)
    Wk = np.asarray(inputs["Wk"], f32)
    Wv = np.asarray(inputs["Wv"], f32)
    # fold LN1 affine into qkv projections
    wq_f = np.ascontiguousarray(g1[:, None] * Wq)
    wk_f = np.ascontiguousarray(g1[:, None] * Wk)
    wv_f = np.ascontiguousarray(g1[:, None] * Wv)
    bq_f = np.asarray(inputs["bq"], f32) + beta1 @ Wq
    bk_f = np.asarray(inputs["bk"], f32) + beta1 @ Wk
    bv_f = np.asarray(inputs["bv"], f32) + beta1 @ Wv
    bqkv = np.ascontiguousarray(np.stack([bq_f, bk_f, bv_f], axis=1))  # [64,3]
    # tile(head, 16) @ Wd == head @ (sum of the 16 row-blocks of Wd)
    Wd_sum = np.ascontiguousarray(
        np.asarray(inputs["Wd"], f32).reshape(N_HEADS, ATTN, EMB).sum(0))
    bd = np.ascontiguousarray(np.asarray(inputs["bd"], f32)[None, :])
    # fold LN2 affine into W1
    W1 = np.asarray(inputs["W1"], f32)
    w1_f = np.ascontiguousarray(g2[:, None] * W1)
    c1_f = np.asarray(inputs["c1"], f32) + beta2 @ W1
    c1_t = np.ascontiguousarray(c1_f.reshape(HC, 128).T)  # [128, HC]
    W2 = np.ascontiguousarray(np.asarray(inputs["W2"], f32))
    c2 = np.ascontiguousarray(np.asarray(inputs["c2"], f32)[None, :])
    Wfc = np.asarray(inputs["Wfc"], f32)
    bfc = np.asarray(inputs["bfc"], f32)

    in_maps = []
    for c in range(N_CORES):
        in_maps.append(dict(
            emb=emb,
            xi=np.ascontiguousarray(x[c * T:(c + 1) * T, None]),
            wq=wq_f, wk=wk_f, wv=wv_f, bqkv=bqkv,
            wd=Wd_sum, bd=bd, w1=w1_f, c1=c1_t, w2=W2, c2=c2,
            wfc=np.ascontiguousarray(Wfc[:, c * VSH:(c + 1) * VSH]),
            bfc=np.ascontiguousarray(bfc[None, c * VSH:(c + 1) * VSH]),
        ))
    return in_maps


def kernel(**inputs) -> np.ndarray:
    nc = _get_nc()
    in_maps = prepare_in_maps(inputs)
    r = run_bass_kernel_spmd(nc, in_maps, core_ids=list(range(N_CORES)))
    logits = np.concatenate([r.results[c]["out"] for c in range(N_CORES)], axis=1)
    return logits.reshape(BATCH, SEQ, VOCAB)


# revision 11
# speedup vs baseline: 1.1954x; 1.1954x over previous
"""Trainium2 Bass kernel for an 8-layer weight-shared decoder stack (v2, fp16).

Model (see problem reference): h = emb[x]; 8x identical decoder layers
(LN -> single-head attn tiled 16x -> proj -> LN -> 4x FFN); fc to vocab.

Distribution over 8 NeuronCores:
  - tokens sharded 8-way (cores 0-3 <- batch 0, cores 4-7 <- batch 1;
    512 tokens per core); per-layer AllGather of K/V within each 4-core
    batch group;
  - final hidden states AllGathered across all 8 cores; fc vocab-sharded
    (4000 columns per core); host concatenates the vocab shards.

Numerics: fp16 matmul operands (11-bit mantissa, same error class as
fp32r but with hideable LDWEIGHTS and FWL), fp32 residual stream and
fp32 PSUM accumulation everywhere.
Algebraic folds: tile(head,16) @ Wd == head @ Wd_sum; LN affine (g, beta)
folded into the following weight matrices; softmax denominator applied
to the AV product instead of the probabilities (linearity).
Activations are stored transposed (embedding on partitions) so no
activation transposes are needed anywhere; attention scores are computed
directly in [key, query] layout and the softmax reductions over keys run
on the PE via ones-vector matmuls.
Large weights (W1/W2/Wfc) are passed pre-swizzled so every tile load is
one contiguous run per partition (no DMA descriptor fragmentation).
"""
import numpy as np
from contextlib import ExitStack

import concourse.bass as bass
import concourse.tile as tile
from concourse import bacc, mybir
from concourse.bass_utils import run_bass_kernel_spmd
from concourse.masks import make_identity

dt = mybir.dt
AF = mybir.ActivationFunctionType
ALU = mybir.AluOpType

# model dims (hardcoded per the problem spec)
VOCAB, EMB, SEQ, STACK, N_HEADS, ATTN, BATCH = 32000, 1024, 2048, 8, 16, 64, 2
N_CORES = 8
T = (BATCH * SEQ) // N_CORES          # 512 tokens per core
GRP = 4                               # cores per batch group
GROUPS = [[0, 1, 2, 3], [4, 5, 6, 7]]
EC = EMB // 128                       # 8 emb chunks
KC = SEQ // 128                       # 16 key chunks (per batch)
HC = 4 * EMB // 128                   # 32 ffn hidden chunks
TC = T // 128                         # 4 local token chunks
VSH = VOCAB // N_CORES                # 4000 vocab per core
VCC = 8                               # vocab col chunks per core
VCW = VSH // VCC                      # 500 cols per chunk
GTC = (BATCH * SEQ) // 128            # 32 global token chunks
F32, I32 = dt.float32, dt.int32
MDT = dt.float16                      # matmul operand dtype
NDT = np.float16


def build_nc():
    nc = bacc.Bacc("TRN2", target_bir_lowering=False, debug=False,
                   enable_asserts=True, num_devices=N_CORES)

    # ---- I/O ----  (w1/w2/wfc are host-swizzled; see prepare_in_maps)
    emb = nc.dram_tensor("emb", [VOCAB, EMB], F32, kind="ExternalInput").ap()
    xi = nc.dram_tensor("xi", [T, 1], I32, kind="ExternalInput").ap()
    wq = nc.dram_tensor("wq", [EMB, ATTN], MDT, kind="ExternalInput").ap()
    wk = nc.dram_tensor("wk", [EMB, ATTN], MDT, kind="ExternalInput").ap()
    wv = nc.dram_tensor("wv", [EMB, ATTN], MDT, kind="ExternalInput").ap()
    bqkv = nc.dram_tensor("bqkv", [ATTN, 3], F32, kind="ExternalInput").ap()
    wd = nc.dram_tensor("wd", [ATTN, EMB], MDT, kind="ExternalInput").ap()  # Wd_sum
    bd = nc.dram_tensor("bd", [1, EMB], MDT, kind="ExternalInput").ap()
    w1 = nc.dram_tensor("w1", [HC, 128, EC * 128], MDT,
                        kind="ExternalInput").ap()          # [hc][p][ec*m]
    c1 = nc.dram_tensor("c1", [128, HC], F32, kind="ExternalInput").ap()
    w2 = nc.dram_tensor("w2", [2, EC, 128, (HC // 2) * 128], MDT,
                        kind="ExternalInput").ap()          # [half][ec][p][j*m]
    c2 = nc.dram_tensor("c2", [1, EMB], MDT, kind="ExternalInput").ap()
    wfc = nc.dram_tensor("wfc", [VCC, 128, EC * VCW], MDT,
                         kind="ExternalInput").ap()         # [vc][p][ec*n]
    bfc = nc.dram_tensor("bfc", [1, VSH], MDT, kind="ExternalInput").ap()
    out = nc.dram_tensor("out", [BATCH * SEQ, VSH], F32, kind="ExternalOutput").ap()

    with tile.TileContext(nc) as tc, ExitStack() as ctx:
        dram = ctx.enter_context(tc.tile_pool(name="dram", bufs=1, space="DRAM"))
        consts = ctx.enter_context(tc.tile_pool(name="consts", bufs=1))
        ps_mm = ctx.enter_context(tc.tile_pool(name="ps_mm", bufs=2, space="PSUM"))
        ps_st = ctx.enter_context(tc.tile_pool(name="ps_st", bufs=2, space="PSUM"))
        ps_v64 = ctx.enter_context(tc.tile_pool(name="ps_v64", bufs=2, space="PSUM"))
        ps_b = ctx.enter_context(tc.tile_pool(name="ps_b", bufs=1, space="PSUM"))

        # ---- constants / small weights resident in SBUF ----
        ident = consts.tile([128, 128], F32, tag="ident")
        make_identity(nc, ident[:])
        identh = consts.tile([64, 64], MDT, tag="identh")
        nc.vector.tensor_copy(identh[:], ident[:64, :64])
        ones_f = consts.tile([128, 1], F32, tag="ones_f")
        nc.vector.memset(ones_f[:], 1.0)
        onesc = consts.tile([128, 1], MDT, tag="onesc")      # ones column
        nc.vector.tensor_copy(onesc[:], ones_f[:])
        ones_rowf = consts.tile([1, T], F32, tag="ones_rowf")
        nc.vector.memset(ones_rowf[:], 1.0)
        onesr = consts.tile([1, T], MDT, tag="onesr")        # ones row
        nc.vector.tensor_copy(onesr[:], ones_rowf[:])
        eps_t = consts.tile([1, 1], F32, tag="eps")
        nc.vector.memset(eps_t[:], 1e-5)

        wq_t = consts.tile([128, EC * ATTN], MDT, tag="wq")
        wk_t = consts.tile([128, EC * ATTN], MDT, tag="wk")
        wv_t = consts.tile([128, EC * ATTN], MDT, tag="wv")
        for w_t, w_d in ((wq_t, wq), (wk_t, wk), (wv_t, wv)):
            nc.sync.dma_start(
                w_t.rearrange("p (ec a) -> p ec a", ec=EC),
                w_d.rearrange("(ec p) a -> p ec a", p=128))
        bqkv_t = consts.tile([ATTN, 3], F32, tag="bqkv")
        nc.sync.dma_start(bqkv_t[:], bqkv)
        wd_t = consts.tile([ATTN, EMB], MDT, tag="wd")
        nc.sync.dma_start(wd_t[:], wd)
        bd_t = consts.tile([1, EMB], MDT, tag="bd")
        nc.sync.dma_start(bd_t[:], bd)
        c1_t = consts.tile([128, HC], F32, tag="c1")
        nc.sync.dma_start(c1_t[:], c1)
        c2_t = consts.tile([1, EMB], MDT, tag="c2")
        nc.sync.dma_start(c2_t[:], c2)

        # final-hidden gather buffers (phase boundary)
        hg_in = dram.tile([EMB, T], MDT, tag="hg_in")
        hg = dram.tile([N_CORES, EMB, T], MDT, tag="hg")

        # ================= phase 1: embed + decoder stack =================
        with ExitStack() as lctx:
            hp = lctx.enter_context(tc.tile_pool(name="hpool", bufs=1))
            lay = lctx.enter_context(tc.tile_pool(name="lay", bufs=2))
            scr = lctx.enter_context(tc.tile_pool(name="scratch", bufs=2))
            abp = lctx.enter_context(tc.tile_pool(name="abp", bufs=1))
            a1p = lctx.enter_context(tc.tile_pool(name="a1p", bufs=1))
            w1p = lctx.enter_context(tc.tile_pool(name="w1p", bufs=4))
            w2p = lctx.enter_context(tc.tile_pool(name="w2p", bufs=2))
            etp = lctx.enter_context(tc.tile_pool(name="etp", bufs=4))
            kvp = lctx.enter_context(tc.tile_pool(name="kvp", bufs=2))
            rows = lctx.enter_context(tc.tile_pool(name="rows", bufs=4))
            rows2 = lctx.enter_context(tc.tile_pool(name="rows2", bufs=2))
            up = lctx.enter_context(tc.tile_pool(name="up", bufs=3))
            embp = lctx.enter_context(tc.tile_pool(name="embp", bufs=2))

            # residual hT: [emb-part, token-free], chunk ec at cols [ec*T,(ec+1)*T)
            h_t = hp.tile([128, EC * T], F32, tag="h")

            def hcol(ec):
                return h_t[:, ec * T:(ec + 1) * T]

            # ---- embedding gather + transpose ----
            with nc.named_scope("embed"):
                for tk in range(TC):
                    idx_t = embp.tile([128, 1], I32, tag="idx")
                    nc.sync.dma_start(idx_t[:], xi[tk * 128:(tk + 1) * 128, :])
                    gat = embp.tile([128, EMB], F32, tag="gat")
                    nc.gpsimd.indirect_dma_start(
                        out=gat[:], out_offset=None, in_=emb,
                        in_offset=bass.IndirectOffsetOnAxis(ap=idx_t[:, :1], axis=0))
                    for ec in range(EC):
                        tr_ps = ps_mm.tile([128, 128], F32, tag="mm")
                        nc.tensor.transpose(
                            tr_ps[:], gat[:, ec * 128:(ec + 1) * 128], ident[:])
                        nc.vector.tensor_copy(
                            h_t[:, ec * T + tk * 128: ec * T + (tk + 1) * 128],
                            tr_ps[:])

            def layernorm(z_t):
                """z = (h - mu(h)) / sqrt(var(h)+eps), fp16 into z_t."""
                hr = scr.tile([128, EC * T], MDT, tag="scr4")
                nc.vector.tensor_copy(hr[:], h_t[:])
                hsq = scr.tile([128, EC * T], MDT, tag="scr4")
                nc.vector.tensor_tensor(hsq[:], h_t[:], h_t[:], op=ALU.mult)
                sum_ps = ps_st.tile([1, T], F32, tag="stat")
                sq_ps = ps_st.tile([1, T], F32, tag="stat")
                for ec in range(EC):
                    nc.tensor.matmul(sum_ps[:], onesc[:], hr[:, ec * T:(ec + 1) * T],
                                     start=(ec == 0), stop=(ec == EC - 1))
                for ec in range(EC):
                    nc.tensor.matmul(sq_ps[:], onesc[:], hsq[:, ec * T:(ec + 1) * T],
                                     start=(ec == 0), stop=(ec == EC - 1))
                nmu = rows.tile([1, T], F32, tag="r1")
                nc.vector.tensor_scalar(nmu[:], sum_ps[:], -1.0 / EMB, None,
                                        op0=ALU.mult)
                var = rows.tile([1, T], F32, tag="r1")
                nc.vector.tensor_scalar(var[:], sq_ps[:], 1.0 / EMB, None,
                                        op0=ALU.mult)
                musq = rows.tile([1, T], F32, tag="r1")
                nc.vector.tensor_tensor(musq[:], nmu[:], nmu[:], op=ALU.mult)
                nc.vector.tensor_tensor(var[:], var[:], musq[:], op=ALU.subtract)
                sd = rows.tile([1, T], F32, tag="r1")
                nc.scalar.activation(sd[:], var[:], AF.Sqrt, bias=eps_t[:])
                istd = rows.tile([1, T], F32, tag="r1")
                nc.vector.reciprocal(istd[:], sd[:])
                ab_row = rows2.tile([1, 2 * T], MDT, tag="r2")
                nc.vector.tensor_copy(ab_row[:, :T], istd[:])
                nc.vector.tensor_tensor(ab_row[:, T:], nmu[:], istd[:], op=ALU.mult)
                ab_ps = ps_b.tile([128, 2 * T], F32, tag="bcast")
                nc.tensor.matmul(ab_ps[:, :T], onesr[:, :128], ab_row[:, :T],
                                 start=True, stop=True)
                nc.tensor.matmul(ab_ps[:, T:], onesr[:, :128], ab_row[:, T:],
                                 start=True, stop=True)
                ab_sb = abp.tile([128, 2 * T], F32, tag="ab")
                nc.vector.tensor_copy(ab_sb[:], ab_ps[:])
                for ec in range(EC):
                    u = up.tile([128, T], F32, tag="u")
                    nc.vector.tensor_tensor(u[:], hcol(ec), ab_sb[:, :T],
                                            op=ALU.mult)
                    nc.vector.tensor_tensor(z_t[:, ec * T:(ec + 1) * T], u[:],
                                            ab_sb[:, T:], op=ALU.add)

            for layer in range(STACK):
                with nc.named_scope(f"L{layer}"):
                    # ---- LN1 + KV first (so the gather launches early) ----
                    z_t = scr.tile([128, EC * T], MDT, tag="scr4")
                    layernorm(z_t)
                    qkv_sb = {}
                    for name, w_t, qi in (("k", wk_t, 1), ("v", wv_t, 2),
                                          ("q", wq_t, 0)):
                        p = ps_v64.tile([ATTN, T], F32, tag="vec64")
                        for ec in range(EC):
                            nc.tensor.matmul(
                                p[:], w_t[:, ec * ATTN:(ec + 1) * ATTN],
                                z_t[:, ec * T:(ec + 1) * T],
                                start=(ec == 0), stop=(ec == EC - 1))
                        s = lay.tile([ATTN, T], MDT, tag=f"qkv{qi}")
                        nc.scalar.activation(s[:], p[:], AF.Identity,
                                             bias=bqkv_t[:, qi:qi + 1])
                        qkv_sb[name] = s
                        if name == "v":
                            # local v -> token-major, then stage k|v and gather
                            v_loc = lay.tile([128, TC * ATTN], MDT, tag="vloc")
                            for tk in range(TC):
                                tp = ps_v64.tile([128, 128], MDT, tag="vec64")
                                nc.tensor.transpose(
                                    tp[:128, :ATTN],
                                    qkv_sb["v"][:, tk * 128:(tk + 1) * 128],
                                    identh[:])
                                nc.vector.tensor_copy(
                                    v_loc[:, tk * ATTN:(tk + 1) * ATTN],
                                    tp[:128, :ATTN])
                            kv_loc = dram.tile([2 * ATTN * T], MDT, tag="kv_loc")
                            nc.sync.dma_start(
                                kv_loc[0:ATTN * T]
                                .rearrange("(a t) -> a t", a=ATTN),
                                qkv_sb["k"][:])
                            nc.sync.dma_start(
                                kv_loc[ATTN * T:].rearrange("(p c) -> p c", p=128),
                                v_loc[:])
                            kv_g = dram.tile([GRP, 2 * ATTN * T], MDT, tag="kv_g")
                            nc.gpsimd.collective_compute(
                                "AllGather", ALU.bypass, replica_groups=GROUPS,
                                ins=[kv_loc.opt()], outs=[kv_g.opt()])
                    qT = qkv_sb["q"]

                    kT = kvp.tile([ATTN, SEQ], MDT, tag="kT")
                    vtm = kvp.tile([128, KC * ATTN], MDT, tag="vtm")
                    for r in range(GRP):
                        nc.sync.dma_start(
                            kT[:, r * T:(r + 1) * T],
                            kv_g[r, 0:ATTN * T].rearrange("(a t) -> a t", a=ATTN))
                        nc.sync.dma_start(
                            vtm[:, r * TC * ATTN:(r + 1) * TC * ATTN]
                            .rearrange("p (c a) -> p c a", c=TC),
                            kv_g[r, ATTN * T:]
                            .rearrange("(p c a) -> p c a", p=128, c=TC))

                    # ---- attention ----
                    # e = exp(scoresT); AV and denominator accumulate per chunk;
                    # 1/denominator is applied to the AV product (linearity)
                    den_ps = ps_st.tile([1, T], F32, tag="stat")
                    head_ps = ps_v64.tile([ATTN, T], F32, tag="vec64")
                    for kc in range(KC):
                        s_ps = ps_mm.tile([128, T], F32, tag="mm")
                        nc.tensor.matmul(s_ps[:], kT[:, kc * 128:(kc + 1) * 128],
                                         qT[:], start=True, stop=True)
                        e_kc = etp.tile([128, T], MDT, tag="eT")
                        nc.scalar.activation(e_kc[:], s_ps[:], AF.Exp,
                                             scale=float(ATTN) ** -0.5)
                        nc.tensor.matmul(den_ps[:], onesc[:], e_kc[:],
                                         start=(kc == 0), stop=(kc == KC - 1))
                        nc.tensor.matmul(head_ps[:],
                                         vtm[:, kc * ATTN:(kc + 1) * ATTN],
                                         e_kc[:],
                                         start=(kc == 0), stop=(kc == KC - 1))
                    recip = rows.tile([1, T], F32, tag="r1")
                    nc.vector.reciprocal(recip[:], den_ps[:])
                    rrow = rows.tile([1, T], MDT, tag="r1")
                    nc.vector.tensor_copy(rrow[:], recip[:])
                    rb_ps = ps_b.tile([128, 2 * T], F32, tag="bcast")
                    nc.tensor.matmul(rb_ps[:ATTN, :T], onesr[:, :ATTN], rrow[:],
                                     start=True, stop=True)
                    rb_sb = abp.tile([ATTN, T], F32, tag="rb")
                    nc.vector.tensor_copy(rb_sb[:], rb_ps[:ATTN, :T])
                    headT = lay.tile([ATTN, T], MDT, tag="headT")
                    nc.vector.tensor_tensor(headT[:], head_ps[:], rb_sb[:],
                                            op=ALU.mult)

                    # ---- proj + residual ----
                    for ec in range(EC):
                        p_ps = ps_mm.tile([128, T], F32, tag="mm")
                        nc.tensor.matmul(p_ps[:], wd_t[:, ec * 128:(ec + 1) * 128],
                                         headT[:], start=True, stop=False)
                        nc.tensor.matmul(p_ps[:], bd_t[:, ec * 128:(ec + 1) * 128],
                                         onesr[:], start=False, stop=True)
                        nc.vector.tensor_tensor(hcol(ec), hcol(ec), p_ps[:],
                                                op=ALU.add)

                    # ---- LN2 + FFN (two half passes over hidden chunks) ----
                    z2_t = scr.tile([128, EC * T], MDT, tag="scr4")
                    layernorm(z2_t)
                    for half in range(2):
                        a1 = a1p.tile([128, (HC // 2) * T], MDT, tag="a1")
                        for j in range(HC // 2):
                            hc = half * (HC // 2) + j
                            w1_t = w1p.tile([128, EC * 128], MDT, tag="w1")
                            nc.sync.dma_start(w1_t[:], w1[hc])
                            f_ps = ps_mm.tile([128, T], F32, tag="mm")
                            for ec in range(EC):
                                nc.tensor.matmul(
                                    f_ps[:], w1_t[:, ec * 128:(ec + 1) * 128],
                                    z2_t[:, ec * T:(ec + 1) * T],
                                    start=(ec == 0), stop=(ec == EC - 1))
                            nc.scalar.activation(a1[:, j * T:(j + 1) * T], f_ps[:],
                                                 AF.Relu, bias=c1_t[:, hc:hc + 1])
                        for ec in range(EC):
                            w2_t = w2p.tile([128, (HC // 2) * 128], MDT, tag="w2")
                            nc.sync.dma_start(w2_t[:], w2[half, ec])
                            g_ps = ps_mm.tile([128, T], F32, tag="mm")
                            for j in range(HC // 2):
                                nc.tensor.matmul(
                                    g_ps[:], w2_t[:, j * 128:(j + 1) * 128],
                                    a1[:, j * T:(j + 1) * T],
                                    start=(j == 0),
                                    stop=(j == HC // 2 - 1 and half == 1))
                            if half == 1:
                                nc.tensor.matmul(
                                    g_ps[:], c2_t[:, ec * 128:(ec + 1) * 128],
                                    onesr[:], start=False, stop=True)
                            nc.vector.tensor_tensor(hcol(ec), hcol(ec), g_ps[:],
                                                    op=ALU.add)

            # ---- round h and stage it for the all-core gather ----
            with nc.named_scope("hgather"):
                hfin = scr.tile([128, EC * T], MDT, tag="scr4")
                nc.vector.tensor_copy(hfin[:], h_t[:])
                for ec in range(EC):
                    nc.sync.dma_start(hg_in[ec * 128:(ec + 1) * 128, :],
                                      hfin[:, ec * T:(ec + 1) * T])

        nc.gpsimd.collective_compute(
            "AllGather", ALU.bypass,
            replica_groups=[list(range(N_CORES))],
            ins=[hg_in.opt()], outs=[hg.opt()])

        # ================= phase 2: fc over the vocab shard =================
        with nc.named_scope("fc"):
            with tc.tile_pool(name="fcp", bufs=1) as fcp, \
                 tc.tile_pool(name="wfcp", bufs=2) as wfcp, \
                 tc.tile_pool(name="outp", bufs=4) as outp, \
                 tc.tile_pool(name="bfcp", bufs=1) as bfcp:
                bfc_t = bfcp.tile([1, VSH], MDT, tag="bfc")
                nc.sync.dma_start(bfc_t[:], bfc)
                hT_full = fcp.tile([128, EC * BATCH * SEQ], MDT, tag="hT")
                for ec in range(EC):
                    for r in range(N_CORES):
                        nc.sync.dma_start(
                            hT_full[:, ec * BATCH * SEQ + r * T:
                                    ec * BATCH * SEQ + (r + 1) * T],
                            hg[r, ec * 128:(ec + 1) * 128, :])
                for vc in range(VCC):
                    wfc_t = wfcp.tile([128, EC * VCW], MDT, tag="wfc")
                    nc.sync.dma_start(wfc_t[:], wfc[vc])
                    for tcg in range(GTC):
                        o_ps = ps_mm.tile([128, VCW], F32, tag="mm")
                        for ec in range(EC):
                            nc.tensor.matmul(
                                o_ps[:],
                                hT_full[:, ec * BATCH * SEQ + tcg * 128:
                                        ec * BATCH * SEQ + (tcg + 1) * 128],
                                wfc_t[:, ec * VCW:(ec + 1) * VCW],
                                start=(ec == 0), stop=False)
                        nc.tensor.matmul(o_ps[:], onesr[:, :128],
                                         bfc_t[:, vc * VCW:(vc + 1) * VCW],
                                         start=False, stop=True)
                        o_sb = outp.tile([128, VCW], F32, tag="osb")
                        nc.vector.tensor_copy(o_sb[:], o_ps[:])
                        nc.sync.dma_start(
                            out[tcg * 128:(tcg + 1) * 128,
                                vc * VCW:(vc + 1) * VCW], o_sb[:])

    nc.compile()
    return nc


_NC_CACHE = None


def _get_nc():
    global _NC_CACHE
    if _NC_CACHE is None:
        _NC_CACHE = build_nc()
    return _NC_CACHE


def prepare_in_maps(inputs):
    f32 = np.float32
    x = np.asarray(inputs["x"]).reshape(-1).astype(np.int32)
    emb = np.ascontiguousarray(np.asarray(inputs["emb"], f32))
    g1 = np.asarray(inputs["g1"], f32)
    beta1 = np.asarray(inputs["beta1"], f32)
    g2 = np.asarray(inputs["g2"], f32)
    beta2 = np.asarray(inputs["beta2"], f32)
    Wq = np.asarray(inputs["Wq"], f32)
    Wk = np.asarray(inputs["Wk"], f32)
    Wv = np.asarray(inputs["Wv"], f32)
    # fold LN1 affine into qkv projections
    wq_f = np.ascontiguousarray((g1[:, None] * Wq).astype(NDT))
    wk_f = np.ascontiguousarray((g1[:, None] * Wk).astype(NDT))
    wv_f = np.ascontiguousarray((g1[:, None] * Wv).astype(NDT))
    bq_f = np.asarray(inputs["bq"], f32) + beta1 @ Wq
    bk_f = np.asarray(inputs["bk"], f32) + beta1 @ Wk
    bv_f = np.asarray(inputs["bv"], f32) + beta1 @ Wv
    bqkv = np.ascontiguousarray(np.stack([bq_f, bk_f, bv_f], axis=1))  # [64,3]
    # tile(head, 16) @ Wd == head @ (sum of the 16 row-blocks of Wd)
    Wd_sum = np.asarray(inputs["Wd"], f32).reshape(N_HEADS, ATTN, EMB).sum(0)
    wd_h = np.ascontiguousarray(Wd_sum.astype(NDT))
    bd = np.ascontiguousarray(np.asarray(inputs["bd"], f32)[None, :].astype(NDT))
    # fold LN2 affine into W1; swizzle to [hc][p][ec*128]
    W1 = np.asarray(inputs["W1"], f32)
    w1_f = (g2[:, None] * W1).astype(NDT)                    # [1024, 4096]
    w1_sw = np.ascontiguousarray(
        w1_f.reshape(EC, 128, HC, 128).transpose(2, 1, 0, 3)
        .reshape(HC, 128, EC * 128))
    c1_f = np.asarray(inputs["c1"], f32) + beta2 @ W1
    c1_t = np.ascontiguousarray(c1_f.reshape(HC, 128).T)     # [128, HC]
    # W2 swizzle to [half][ec][p][j*128]
    W2 = np.asarray(inputs["W2"], f32).astype(NDT)           # [4096, 1024]
    w2_sw = np.ascontiguousarray(
        W2.reshape(2, HC // 2, 128, EC, 128).transpose(0, 3, 2, 1, 4)
        .reshape(2, EC, 128, (HC // 2) * 128))
    c2 = np.ascontiguousarray(np.asarray(inputs["c2"], f32)[None, :].astype(NDT))
    Wfc = np.asarray(inputs["Wfc"], f32)
    bfc = np.asarray(inputs["bfc"], f32)

    in_maps = []
    for c in range(N_CORES):
        wfc_c = Wfc[:, c * VSH:(c + 1) * VSH].astype(NDT)    # [1024, 4000]
        wfc_sw = np.ascontiguousarray(
            wfc_c.reshape(EC, 128, VCC, VCW).transpose(2, 1, 0, 3)
            .reshape(VCC, 128, EC * VCW))
        in_maps.append(dict(
            emb=emb,
            xi=np.ascontiguousarray(x[c * T:(c + 1) * T, None]),
            wq=wq_f, wk=wk_f, wv=wv_f, bqkv=bqkv,
            wd=wd_h, bd=bd, w1=w1_sw, c1=c1_t, w2=w2_sw, c2=c2,
            wfc=wfc_sw,
            bfc=np.ascontiguousarray(bfc[None, c * VSH:(c + 1) * VSH].astype(NDT)),
        ))
    return in_maps


def kernel(**inputs) -> np.ndarray:
    nc = _get_nc()
    in_maps = prepare_in_maps(inputs)
    r = run_bass_kernel_spmd(nc, in_maps, core_ids=list(range(N_CORES)))
    logits = np.concatenate([r.results[c]["out"] for c in range(N_CORES)], axis=1)
    return logits.reshape(BATCH, SEQ, VOCAB)


# revision 13
# speedup vs baseline: 1.2435x; 1.0402x over previous
"""Trainium2 Bass kernel for an 8-layer weight-shared decoder stack (v2, fp16).

Model (see problem reference): h = emb[x]; 8x identical decoder layers
(LN -> single-head attn tiled 16x -> proj -> LN -> 4x FFN); fc to vocab.

Distribution over 8 NeuronCores:
  - tokens sharded 8-way (cores 0-3 <- batch 0, cores 4-7 <- batch 1;
    512 tokens per core); per-layer AllGather of K/V within each 4-core
    batch group;
  - final hidden states AllGathered across all 8 cores; fc vocab-sharded
    (4000 columns per core); host concatenates the vocab shards.

Numerics: fp16 matmul operands (11-bit mantissa, same error class as
fp32r but with hideable LDWEIGHTS and FWL), fp32 residual stream and
fp32 PSUM accumulation everywhere.
Algebraic folds: tile(head,16) @ Wd == head @ Wd_sum; LN affine (g, beta)
folded into the following weight matrices; softmax denominator applied
to the AV product instead of the probabilities (linearity).
Activations are stored transposed (embedding on partitions) so no
activation transposes are needed anywhere; attention scores are computed
directly in [key, query] layout and the softmax reductions over keys run
on the PE via ones-vector matmuls.
Large weights (W1/W2/Wfc) are passed pre-swizzled so every tile load is
one contiguous run per partition (no DMA descriptor fragmentation).
"""
import numpy as np
from contextlib import ExitStack

import concourse.bass as bass
import concourse.tile as tile
from concourse import bacc, mybir
from concourse.bass_utils import run_bass_kernel_spmd
from concourse.masks import make_identity

dt = mybir.dt
AF = mybir.ActivationFunctionType
ALU = mybir.AluOpType

# model dims (hardcoded per the problem spec)
VOCAB, EMB, SEQ, STACK, N_HEADS, ATTN, BATCH = 32000, 1024, 2048, 8, 16, 64, 2
N_CORES = 8
T = (BATCH * SEQ) // N_CORES          # 512 tokens per core
GRP = 4                               # cores per batch group
GROUPS = [[0, 1, 2, 3], [4, 5, 6, 7]]
EC = EMB // 128                       # 8 emb chunks
KC = SEQ // 128                       # 16 key chunks (per batch)
HC = 4 * EMB // 128                   # 32 ffn hidden chunks
TC = T // 128                         # 4 local token chunks
VSH = VOCAB // N_CORES                # 4000 vocab per core
VCC = 8                               # vocab col chunks per core
VCW = VSH // VCC                      # 500 cols per chunk
GTC = (BATCH * SEQ) // 128            # 32 global token chunks
F32, I32 = dt.float32, dt.int32
MDT = dt.float16                      # matmul operand dtype
NDT = np.float16


def build_nc():
    nc = bacc.Bacc("TRN2", target_bir_lowering=False, debug=False,
                   enable_asserts=True, num_devices=N_CORES)

    # ---- I/O ----  (w1/w2/wfc are host-swizzled; see prepare_in_maps)
    emb = nc.dram_tensor("emb", [VOCAB, EMB], F32, kind="ExternalInput").ap()
    xi = nc.dram_tensor("xi", [T, 1], I32, kind="ExternalInput").ap()
    wq = nc.dram_tensor("wq", [EMB, ATTN], MDT, kind="ExternalInput").ap()
    wk = nc.dram_tensor("wk", [EMB, ATTN], MDT, kind="ExternalInput").ap()
    wv = nc.dram_tensor("wv", [EMB, ATTN], MDT, kind="ExternalInput").ap()
    bqkv = nc.dram_tensor("bqkv", [ATTN, 3], F32, kind="ExternalInput").ap()
    wd = nc.dram_tensor("wd", [ATTN, EMB], MDT, kind="ExternalInput").ap()  # Wd_sum
    bd = nc.dram_tensor("bd", [1, EMB], MDT, kind="ExternalInput").ap()
    w1 = nc.dram_tensor("w1", [HC, 128, EC * 128], MDT,
                        kind="ExternalInput").ap()          # [hc][p][ec*m]
    c1 = nc.dram_tensor("c1", [128, HC], F32, kind="ExternalInput").ap()
    w2 = nc.dram_tensor("w2", [2, EC, 128, (HC // 2) * 128], MDT,
                        kind="ExternalInput").ap()          # [half][ec][p][j*m]
    c2 = nc.dram_tensor("c2", [1, EMB], MDT, kind="ExternalInput").ap()
    wfc = nc.dram_tensor("wfc", [VOCAB // VCW, 128, EC * VCW], MDT,
                         kind="ExternalInput").ap()         # [vc][p][ec*n]
    bfc = nc.dram_tensor("bfc", [VOCAB // VCW, VCW], MDT, kind="ExternalInput").ap()
    mbias = nc.dram_tensor("mbias", [128, GRP], F32, kind="ExternalInput").ap()
    out = nc.dram_tensor("out", [T, VOCAB], F32, kind="ExternalOutput").ap()

    with tile.TileContext(nc) as tc, ExitStack() as ctx:
        dram = ctx.enter_context(tc.tile_pool(name="dram", bufs=1, space="DRAM"))
        consts = ctx.enter_context(tc.tile_pool(name="consts", bufs=1))
        ps_mm = ctx.enter_context(tc.tile_pool(name="ps_mm", bufs=2, space="PSUM"))
        ps_st = ctx.enter_context(tc.tile_pool(name="ps_st", bufs=2, space="PSUM"))
        ps_v64 = ctx.enter_context(tc.tile_pool(name="ps_v64", bufs=2, space="PSUM"))
        ps_b = ctx.enter_context(tc.tile_pool(name="ps_b", bufs=1, space="PSUM"))

        # ---- constants / small weights resident in SBUF ----
        ident = consts.tile([128, 128], F32, tag="ident")
        make_identity(nc, ident[:])
        identh = consts.tile([64, 64], MDT, tag="identh")
        nc.vector.tensor_copy(identh[:], ident[:64, :64])
        ones_f = consts.tile([128, 1], F32, tag="ones_f")
        nc.vector.memset(ones_f[:], 1.0)
        onesc = consts.tile([128, 1], MDT, tag="onesc")      # ones column
        nc.vector.tensor_copy(onesc[:], ones_f[:])
        ones_rowf = consts.tile([1, T], F32, tag="ones_rowf")
        nc.vector.memset(ones_rowf[:], 1.0)
        onesr = consts.tile([1, T], MDT, tag="onesr")        # ones row
        nc.vector.tensor_copy(onesr[:], ones_rowf[:])
        eps_t = consts.tile([1, 1], F32, tag="eps")
        nc.vector.memset(eps_t[:], 1e-5)
        zbias = consts.tile([128, 1], F32, tag="zbias")
        nc.vector.memset(zbias[:], 0.0)
        mbias_t = consts.tile([128, GRP], F32, tag="mbias")
        nc.sync.dma_start(mbias_t[:], mbias)

        wq_t = consts.tile([128, EC * ATTN], MDT, tag="wq")
        wk_t = consts.tile([128, EC * ATTN], MDT, tag="wk")
        wv_t = consts.tile([128, EC * ATTN], MDT, tag="wv")
        for w_t, w_d in ((wq_t, wq), (wk_t, wk), (wv_t, wv)):
            nc.sync.dma_start(
                w_t.rearrange("p (ec a) -> p ec a", ec=EC),
                w_d.rearrange("(ec p) a -> p ec a", p=128))
        bqkv_t = consts.tile([ATTN, 3], F32, tag="bqkv")
        nc.sync.dma_start(bqkv_t[:], bqkv)
        wd_t = consts.tile([ATTN, EMB], MDT, tag="wd")
        nc.sync.dma_start(wd_t[:], wd)
        bd_t = consts.tile([1, EMB], MDT, tag="bd")
        nc.sync.dma_start(bd_t[:], bd)
        c1_t = consts.tile([128, HC], F32, tag="c1")
        nc.sync.dma_start(c1_t[:], c1)
        c2_t = consts.tile([1, EMB], MDT, tag="c2")
        nc.sync.dma_start(c2_t[:], c2)

        # final hidden (fp16) handed from phase 1 to the fc phase
        hfp = ctx.enter_context(tc.tile_pool(name="hfp", bufs=1))
        hfin = hfp.tile([128, EC * T], MDT, tag="hfin")

        # ================= phase 1: embed + decoder stack =================
        with ExitStack() as lctx:
            hp = lctx.enter_context(tc.tile_pool(name="hpool", bufs=1))
            lay = lctx.enter_context(tc.tile_pool(name="lay", bufs=2))
            scr = lctx.enter_context(tc.tile_pool(name="scratch", bufs=2))
            abp = lctx.enter_context(tc.tile_pool(name="abp", bufs=1))
            a1p = lctx.enter_context(tc.tile_pool(name="a1p", bufs=1))
            w1p = lctx.enter_context(tc.tile_pool(name="w1p", bufs=4))
            w2p = lctx.enter_context(tc.tile_pool(name="w2p", bufs=2))
            etp = lctx.enter_context(tc.tile_pool(name="etp", bufs=4))
            kvp = lctx.enter_context(tc.tile_pool(name="kvp", bufs=2))
            rows = lctx.enter_context(tc.tile_pool(name="rows", bufs=4))
            rows2 = lctx.enter_context(tc.tile_pool(name="rows2", bufs=2))
            up = lctx.enter_context(tc.tile_pool(name="up", bufs=3))
            embp = lctx.enter_context(tc.tile_pool(name="embp", bufs=2))

            # residual hT: [emb-part, token-free], chunk ec at cols [ec*T,(ec+1)*T)
            h_t = hp.tile([128, EC * T], F32, tag="h")

            def hcol(ec):
                return h_t[:, ec * T:(ec + 1) * T]

            # ---- embedding gather + transpose ----
            with nc.named_scope("embed"):
                for tk in range(TC):
                    idx_t = embp.tile([128, 1], I32, tag="idx")
                    nc.sync.dma_start(idx_t[:], xi[tk * 128:(tk + 1) * 128, :])
                    gat = embp.tile([128, EMB], F32, tag="gat")
                    nc.gpsimd.indirect_dma_start(
                        out=gat[:], out_offset=None, in_=emb,
                        in_offset=bass.IndirectOffsetOnAxis(ap=idx_t[:, :1], axis=0))
                    for ec in range(EC):
                        tr_ps = ps_mm.tile([128, 128], F32, tag="mm")
                        nc.tensor.transpose(
                            tr_ps[:], gat[:, ec * 128:(ec + 1) * 128], ident[:])
                        nc.vector.tensor_copy(
                            h_t[:, ec * T + tk * 128: ec * T + (tk + 1) * 128],
                            tr_ps[:])

            def layernorm(z_t):
                """z = (h - mu(h)) / sqrt(var(h)+eps), fp16 into z_t."""
                hr = scr.tile([128, EC * T], MDT, tag="scr4")
                nc.vector.tensor_copy(hr[:], h_t[:])
                hsq = scr.tile([128, EC * T], MDT, tag="scr4")
                nc.vector.tensor_tensor(hsq[:], h_t[:], h_t[:], op=ALU.mult)
                sum_ps = ps_st.tile([1, T], F32, tag="stat")
                sq_ps = ps_st.tile([1, T], F32, tag="stat")
                for ec in range(EC):
                    nc.tensor.matmul(sum_ps[:], onesc[:], hr[:, ec * T:(ec + 1) * T],
                                     start=(ec == 0), stop=(ec == EC - 1))
                for ec in range(EC):
                    nc.tensor.matmul(sq_ps[:], onesc[:], hsq[:, ec * T:(ec + 1) * T],
                                     start=(ec == 0), stop=(ec == EC - 1))
                nmu = rows.tile([1, T], F32, tag="r1")
                nc.vector.tensor_scalar(nmu[:], sum_ps[:], -1.0 / EMB, None,
                                        op0=ALU.mult)
                var = rows.tile([1, T], F32, tag="r1")
                nc.vector.tensor_scalar(var[:], sq_ps[:], 1.0 / EMB, None,
                                        op0=ALU.mult)
                musq = rows.tile([1, T], F32, tag="r1")
                nc.vector.tensor_tensor(musq[:], nmu[:], nmu[:], op=ALU.mult)
                nc.vector.tensor_tensor(var[:], var[:], musq[:], op=ALU.subtract)
                sd = rows.tile([1, T], F32, tag="r1")
                nc.scalar.activation(sd[:], var[:], AF.Sqrt, bias=eps_t[:])
                istd = rows.tile([1, T], F32, tag="r1")
                nc.vector.reciprocal(istd[:], sd[:])
                ab_row = rows2.tile([1, 2 * T], MDT, tag="r2")
                nc.vector.tensor_copy(ab_row[:, :T], istd[:])
                nc.vector.tensor_tensor(ab_row[:, T:], nmu[:], istd[:], op=ALU.mult)
                ab_ps = ps_b.tile([128, 2 * T], F32, tag="bcast")
                nc.tensor.matmul(ab_ps[:, :T], onesr[:, :128], ab_row[:, :T],
                                 start=True, stop=True)
                nc.tensor.matmul(ab_ps[:, T:], onesr[:, :128], ab_row[:, T:],
                                 start=True, stop=True)
                ab_sb = abp.tile([128, 2 * T], F32, tag="ab")
                nc.vector.tensor_copy(ab_sb[:], ab_ps[:])
                for ec in range(EC):
                    u = up.tile([128, T], F32, tag="u")
                    nc.vector.tensor_tensor(u[:], hcol(ec), ab_sb[:, :T],
                                            op=ALU.mult)
                    nc.vector.tensor_tensor(z_t[:, ec * T:(ec + 1) * T], u[:],
                                            ab_sb[:, T:], op=ALU.add)

            for layer in range(STACK):
                with nc.named_scope(f"L{layer}"):
                    # ---- LN1 + KV first (so the gather launches early) ----
                    z_t = scr.tile([128, EC * T], MDT, tag="scr4")
                    layernorm(z_t)
                    qkv_sb = {}
                    for name, w_t, qi in (("k", wk_t, 1), ("v", wv_t, 2),
                                          ("q", wq_t, 0)):
                        p = ps_v64.tile([ATTN, T], F32, tag="vec64")
                        for ec in range(EC):
                            nc.tensor.matmul(
                                p[:], w_t[:, ec * ATTN:(ec + 1) * ATTN],
                                z_t[:, ec * T:(ec + 1) * T],
                                start=(ec == 0), stop=(ec == EC - 1))
                        s = lay.tile([ATTN, T], MDT, tag=f"qkv{qi}")
                        nc.scalar.activation(s[:], p[:], AF.Identity,
                                             bias=bqkv_t[:, qi:qi + 1])
                        qkv_sb[name] = s
                        if name == "v":
                            # local v -> token-major, then stage k|v and gather
                            v_loc = lay.tile(
                                [128, TC * ATTN], MDT, tag="vloc")
                            qkv_sb["vloc"] = v_loc
                            for tk in range(TC):
                                tp = ps_v64.tile([128, 128], MDT, tag="vec64")
                                nc.tensor.transpose(
                                    tp[:128, :ATTN],
                                    qkv_sb["v"][:, tk * 128:(tk + 1) * 128],
                                    identh[:])
                                nc.vector.tensor_copy(
                                    v_loc[:, tk * ATTN:(tk + 1) * ATTN],
                                    tp[:128, :ATTN])
                            kv_loc = dram.tile([2 * ATTN * T], MDT, tag="kv_loc")
                            nc.sync.dma_start(
                                kv_loc[0:ATTN * T]
                                .rearrange("(a t) -> a t", a=ATTN),
                                qkv_sb["k"][:])
                            nc.sync.dma_start(
                                kv_loc[ATTN * T:].rearrange("(p c) -> p c", p=128),
                                v_loc[:])
                            kv_g = dram.tile([GRP, 2 * ATTN * T], MDT, tag="kv_g")
                            nc.gpsimd.collective_compute(
                                "AllGather", ALU.bypass, replica_groups=GROUPS,
                                ins=[kv_loc.opt()], outs=[kv_g.opt()])
                    qT = qkv_sb["q"]

                    kT = kvp.tile([ATTN, SEQ], MDT, tag="kT")
                    vtm = kvp.tile([128, KC * ATTN], MDT, tag="vtm")
                    for r in range(GRP):
                        nc.sync.dma_start(
                            kT[:, r * T:(r + 1) * T],
                            kv_g[r, 0:ATTN * T].rearrange("(a t) -> a t", a=ATTN))
                        nc.sync.dma_start(
                            vtm[:, r * TC * ATTN:(r + 1) * TC * ATTN]
                            .rearrange("p (c a) -> p c a", c=TC),
                            kv_g[r, ATTN * T:]
                            .rearrange("(p c a) -> p c a", p=128, c=TC))

                    # ---- attention ----
                    # e = exp(scoresT); AV and denominator accumulate per chunk;
                    # 1/denominator is applied to the AV product (linearity).
                    # The core's own quarter runs from local tiles while the
                    # gather is in flight; the gathered copy of that quarter is
                    # zeroed via an exp bias of -1e4 (same program on all cores,
                    # mask supplied per core).
                    den_ps = ps_st.tile([1, T], F32, tag="stat")
                    head_ps = ps_v64.tile([ATTN, T], F32, tag="vec64")
                    scale = float(ATTN) ** -0.5
                    for lk in range(TC):
                        s_ps = ps_mm.tile([128, T], F32, tag="mm")
                        nc.tensor.matmul(s_ps[:],
                                         qkv_sb["k"][:, lk * 128:(lk + 1) * 128],
                                         qT[:], start=True, stop=True)
                        e_kc = etp.tile([128, T], MDT, tag="eT")
                        nc.scalar.activation(e_kc[:], s_ps[:], AF.Exp,
                                             scale=scale, bias=zbias[:, :1])
                        nc.tensor.matmul(den_ps[:], onesc[:], e_kc[:],
                                         start=(lk == 0), stop=False)
                        nc.tensor.matmul(head_ps[:],
                                         qkv_sb["vloc"][:, lk * ATTN:(lk + 1) * ATTN],
                                         e_kc[:], start=(lk == 0), stop=False)
                    for kc in range(KC):
                        r = kc // TC
                        s_ps = ps_mm.tile([128, T], F32, tag="mm")
                        nc.tensor.matmul(s_ps[:], kT[:, kc * 128:(kc + 1) * 128],
                                         qT[:], start=True, stop=True)
                        e_kc = etp.tile([128, T], MDT, tag="eT")
                        nc.scalar.activation(e_kc[:], s_ps[:], AF.Exp,
                                             scale=scale, bias=mbias_t[:, r:r + 1])
                        nc.tensor.matmul(den_ps[:], onesc[:], e_kc[:],
                                         start=False, stop=(kc == KC - 1))
                        nc.tensor.matmul(head_ps[:],
                                         vtm[:, kc * ATTN:(kc + 1) * ATTN],
                                         e_kc[:],
                                         start=False, stop=(kc == KC - 1))
                    recip = rows.tile([1, T], F32, tag="r1")
                    nc.vector.reciprocal(recip[:], den_ps[:])
                    rrow = rows.tile([1, T], MDT, tag="r1")
                    nc.vector.tensor_copy(rrow[:], recip[:])
                    rb_ps = ps_b.tile([128, 2 * T], F32, tag="bcast")
                    nc.tensor.matmul(rb_ps[:ATTN, :T], onesr[:, :ATTN], rrow[:],
                                     start=True, stop=True)
                    rb_sb = abp.tile([ATTN, T], F32, tag="rb")
                    nc.vector.tensor_copy(rb_sb[:], rb_ps[:ATTN, :T])
                    headT = lay.tile([ATTN, T], MDT, tag="headT")
                    nc.vector.tensor_tensor(headT[:], head_ps[:], rb_sb[:],
                                            op=ALU.mult)

                    # ---- proj + residual ----
                    for ec in range(EC):
                        p_ps = ps_mm.tile([128, T], F32, tag="mm")
                        nc.tensor.matmul(p_ps[:], wd_t[:, ec * 128:(ec + 1) * 128],
                                         headT[:], start=True, stop=False)
                        nc.tensor.matmul(p_ps[:], bd_t[:, ec * 128:(ec + 1) * 128],
                                         onesr[:], start=False, stop=True)
                        nc.vector.tensor_tensor(hcol(ec), hcol(ec), p_ps[:],
                                                op=ALU.add)

                    # ---- LN2 + FFN (two half passes over hidden chunks) ----
                    z2_t = scr.tile([128, EC * T], MDT, tag="scr4")
                    layernorm(z2_t)
                    for half in range(2):
                        a1 = a1p.tile([128, (HC // 2) * T], MDT, tag="a1")
                        for j in range(HC // 2):
                            hc = half * (HC // 2) + j
                            w1_t = w1p.tile([128, EC * 128], MDT, tag="w1")
                            nc.sync.dma_start(w1_t[:], w1[hc])
                            f_ps = ps_mm.tile([128, T], F32, tag="mm")
                            for ec in range(EC):
                                nc.tensor.matmul(
                                    f_ps[:], w1_t[:, ec * 128:(ec + 1) * 128],
                                    z2_t[:, ec * T:(ec + 1) * T],
                                    start=(ec == 0), stop=(ec == EC - 1))
                            nc.scalar.activation(a1[:, j * T:(j + 1) * T], f_ps[:],
                                                 AF.Relu, bias=c1_t[:, hc:hc + 1])
                        for ec in range(EC):
                            w2_t = w2p.tile([128, (HC // 2) * 128], MDT, tag="w2")
                            nc.sync.dma_start(w2_t[:], w2[half, ec])
                            g_ps = ps_mm.tile([128, T], F32, tag="mm")
                            for j in range(HC // 2):
                                nc.tensor.matmul(
                                    g_ps[:], w2_t[:, j * 128:(j + 1) * 128],
                                    a1[:, j * T:(j + 1) * T],
                                    start=(j == 0),
                                    stop=(j == HC // 2 - 1 and half == 1))
                            if half == 1:
                                nc.tensor.matmul(
                                    g_ps[:], c2_t[:, ec * 128:(ec + 1) * 128],
                                    onesr[:], start=False, stop=True)
                            nc.vector.tensor_tensor(hcol(ec), hcol(ec), g_ps[:],
                                                    op=ALU.add)

            # ---- final hidden to fp16 for the local-token fc ----
            with nc.named_scope("hfin"):
                nc.vector.tensor_copy(hfin[:], h_t[:])

        # ======= phase 2: fc, local tokens x full vocab (no collective) =======
        with nc.named_scope("fc"):
            with tc.tile_pool(name="wfcp", bufs=3) as wfcp, \
                 tc.tile_pool(name="outp", bufs=4) as outp, \
                 tc.tile_pool(name="bfcp", bufs=2) as bfcp:
                NVC = VOCAB // VCW
                for vc in range(NVC):
                    wfc_t = wfcp.tile([128, EC * VCW], MDT, tag="wfc")
                    nc.sync.dma_start(wfc_t[:], wfc[vc])
                    bfc_t = bfcp.tile([1, VCW], MDT, tag="bfc")
                    nc.sync.dma_start(bfc_t[:], bfc[vc:vc + 1, :])
                    for tcg in range(TC):
                        o_ps = ps_mm.tile([128, VCW], F32, tag="mm")
                        for ec in range(EC):
                            nc.tensor.matmul(
                                o_ps[:],
                                hfin[:, ec * T + tcg * 128:
                                     ec * T + (tcg + 1) * 128],
                                wfc_t[:, ec * VCW:(ec + 1) * VCW],
                                start=(ec == 0), stop=False)
                        nc.tensor.matmul(o_ps[:], onesr[:, :128], bfc_t[:],
                                         start=False, stop=True)
                        o_sb = outp.tile([128, VCW], F32, tag="osb")
                        nc.vector.tensor_copy(o_sb[:], o_ps[:])
                        nc.sync.dma_start(
                            out[tcg * 128:(tcg + 1) * 128,
                                vc * VCW:(vc + 1) * VCW], o_sb[:])

    nc.compile()
    return nc


_NC_CACHE = None


def _get_nc():
    global _NC_CACHE
    if _NC_CACHE is None:
        _NC_CACHE = build_nc()
    return _NC_CACHE


def prepare_in_maps(inputs):
    f32 = np.float32
    x = np.asarray(inputs["x"]).reshape(-1).astype(np.int32)
    emb = np.ascontiguousarray(np.asarray(inputs["emb"], f32))
    g1 = np.asarray(inputs["g1"], f32)
    beta1 = np.asarray(inputs["beta1"], f32)
    g2 = np.asarray(inputs["g2"], f32)
    beta2 = np.asarray(inputs["beta2"], f32)
    Wq = np.asarray(inputs["Wq"], f32)
    Wk = np.asarray(inputs["Wk"], f32)
    Wv = np.asarray(inputs["Wv"], f32)
    # fold LN1 affine into qkv projections
    wq_f = np.ascontiguousarray((g1[:, None] * Wq).astype(NDT))
    wk_f = np.ascontiguousarray((g1[:, None] * Wk).astype(NDT))
    wv_f = np.ascontiguousarray((g1[:, None] * Wv).astype(NDT))
    bq_f = np.asarray(inputs["bq"], f32) + beta1 @ Wq
    bk_f = np.asarray(inputs["bk"], f32) + beta1 @ Wk
    bv_f = np.asarray(inputs["bv"], f32) + beta1 @ Wv
    bqkv = np.ascontiguousarray(np.stack([bq_f, bk_f, bv_f], axis=1))  # [64,3]
    # tile(head, 16) @ Wd == head @ (sum of the 16 row-blocks of Wd)
    Wd_sum = np.asarray(inputs["Wd"], f32).reshape(N_HEADS, ATTN, EMB).sum(0)
    wd_h = np.ascontiguousarray(Wd_sum.astype(NDT))
    bd = np.ascontiguousarray(np.asarray(inputs["bd"], f32)[None, :].astype(NDT))
    # fold LN2 affine into W1; swizzle to [hc][p][ec*128]
    W1 = np.asarray(inputs["W1"], f32)
    w1_f = (g2[:, None] * W1).astype(NDT)                    # [1024, 4096]
    w1_sw = np.ascontiguousarray(
        w1_f.reshape(EC, 128, HC, 128).transpose(2, 1, 0, 3)
        .reshape(HC, 128, EC * 128))
    c1_f = np.asarray(inputs["c1"], f32) + beta2 @ W1
    c1_t = np.ascontiguousarray(c1_f.reshape(HC, 128).T)     # [128, HC]
    # W2 swizzle to [half][ec][p][j*128]
    W2 = np.asarray(inputs["W2"], f32).astype(NDT)           # [4096, 1024]
    w2_sw = np.ascontiguousarray(
        W2.reshape(2, HC // 2, 128, EC, 128).transpose(0, 3, 2, 1, 4)
        .reshape(2, EC, 128, (HC // 2) * 128))
    c2 = np.ascontiguousarray(np.asarray(inputs["c2"], f32)[None, :].astype(NDT))
    Wfc = np.asarray(inputs["Wfc"], f32)
    bfc = np.asarray(inputs["bfc"], f32)

    NVC = VOCAB // VCW
    wfc_sw = np.ascontiguousarray(
        Wfc.astype(NDT).reshape(EC, 128, NVC, VCW).transpose(2, 1, 0, 3)
        .reshape(NVC, 128, EC * VCW))
    bfc_sw = np.ascontiguousarray(bfc.astype(NDT).reshape(NVC, VCW))
    in_maps = []
    for c in range(N_CORES):
        mb = np.zeros((128, GRP), np.float32)
        mb[:, c % GRP] = -1e4
        in_maps.append(dict(
            emb=emb,
            xi=np.ascontiguousarray(x[c * T:(c + 1) * T, None]),
            wq=wq_f, wk=wk_f, wv=wv_f, bqkv=bqkv,
            wd=wd_h, bd=bd, w1=w1_sw, c1=c1_t, w2=w2_sw, c2=c2,
            wfc=wfc_sw, bfc=bfc_sw, mbias=mb,
        ))
    return in_maps


def kernel(**inputs) -> np.ndarray:
    nc = _get_nc()
    in_maps = prepare_in_maps(inputs)
    r = run_bass_kernel_spmd(nc, in_maps, core_ids=list(range(N_CORES)))
    logits = np.concatenate([r.results[c]["out"] for c in range(N_CORES)], axis=0)
    return logits.reshape(BATCH, SEQ, VOCAB)


# revision 15
# speedup vs baseline: 1.2565x; 1.0105x over previous
"""Trainium2 Bass kernel for an 8-layer weight-shared decoder stack (v2, fp16).

Model (see problem reference): h = emb[x]; 8x identical decoder layers
(LN -> single-head attn tiled 16x -> proj -> LN -> 4x FFN); fc to vocab.

Distribution over 8 NeuronCores:
  - tokens sharded 8-way (cores 0-3 <- batch 0, cores 4-7 <- batch 1;
    512 tokens per core); per-layer AllGather of K/V within each 4-core
    batch group;
  - final hidden states AllGathered across all 8 cores; fc vocab-sharded
    (4000 columns per core); host concatenates the vocab shards.

Numerics: fp16 matmul operands (11-bit mantissa, same error class as
fp32r but with hideable LDWEIGHTS and FWL), fp32 residual stream and
fp32 PSUM accumulation everywhere.
Algebraic folds: tile(head,16) @ Wd == head @ Wd_sum; LN affine (g, beta)
folded into the following weight matrices; softmax denominator applied
to the AV product instead of the probabilities (linearity).
Activations are stored transposed (embedding on partitions) so no
activation transposes are needed anywhere; attention scores are computed
directly in [key, query] layout and the softmax reductions over keys run
on the PE via ones-vector matmuls.
Large weights (W1/W2/Wfc) are passed pre-swizzled so every tile load is
one contiguous run per partition (no DMA descriptor fragmentation).
"""
import numpy as np
from contextlib import ExitStack

import concourse.bass as bass
import concourse.tile as tile
from concourse import bacc, mybir
from concourse.bass_utils import run_bass_kernel_spmd
from concourse.masks import make_identity

dt = mybir.dt
AF = mybir.ActivationFunctionType
ALU = mybir.AluOpType

# model dims (hardcoded per the problem spec)
VOCAB, EMB, SEQ, STACK, N_HEADS, ATTN, BATCH = 32000, 1024, 2048, 8, 16, 64, 2
N_CORES = 8
T = (BATCH * SEQ) // N_CORES          # 512 tokens per core
GRP = 4                               # cores per batch group
GROUPS = [[0, 1, 2, 3], [4, 5, 6, 7]]
EC = EMB // 128                       # 8 emb chunks
KC = SEQ // 128                       # 16 key chunks (per batch)
HC = 4 * EMB // 128                   # 32 ffn hidden chunks
TC = T // 128                         # 4 local token chunks
VSH = VOCAB // N_CORES                # 4000 vocab per core
VCC = 8                               # vocab col chunks per core
VCW = VSH // VCC                      # 500 cols per chunk
GTC = (BATCH * SEQ) // 128            # 32 global token chunks
F32, I32 = dt.float32, dt.int32
MDT = dt.float16                      # matmul operand dtype
NDT = np.float16


def build_nc():
    nc = bacc.Bacc("TRN2", target_bir_lowering=False, debug=False,
                   enable_asserts=True, num_devices=N_CORES)

    # ---- I/O ----  (w1/w2/wfc are host-swizzled; see prepare_in_maps)
    emb = nc.dram_tensor("emb", [VOCAB, EMB], F32, kind="ExternalInput").ap()
    xi = nc.dram_tensor("xi", [T, 1], I32, kind="ExternalInput").ap()
    wq = nc.dram_tensor("wq", [EMB, ATTN], MDT, kind="ExternalInput").ap()
    wk = nc.dram_tensor("wk", [EMB, ATTN], MDT, kind="ExternalInput").ap()
    wv = nc.dram_tensor("wv", [EMB, ATTN], MDT, kind="ExternalInput").ap()
    bqkv = nc.dram_tensor("bqkv", [ATTN, 3], F32, kind="ExternalInput").ap()
    wd = nc.dram_tensor("wd", [ATTN, EMB], MDT, kind="ExternalInput").ap()  # Wd_sum
    bd = nc.dram_tensor("bd", [1, EMB], MDT, kind="ExternalInput").ap()
    w1 = nc.dram_tensor("w1", [HC, 128, EC * 128], MDT,
                        kind="ExternalInput").ap()          # [hc][p][ec*m]
    c1 = nc.dram_tensor("c1", [128, HC], F32, kind="ExternalInput").ap()
    w2 = nc.dram_tensor("w2", [2, EC, 128, (HC // 2) * 128], MDT,
                        kind="ExternalInput").ap()          # [half][ec][p][j*m]
    c2 = nc.dram_tensor("c2", [1, EMB], MDT, kind="ExternalInput").ap()
    wfc = nc.dram_tensor("wfc", [VOCAB // VCW, 128, EC * VCW], MDT,
                         kind="ExternalInput").ap()         # [vc][p][ec*n]
    bfc = nc.dram_tensor("bfc", [VOCAB // VCW, VCW], MDT, kind="ExternalInput").ap()
    mbias = nc.dram_tensor("mbias", [128, GRP], F32, kind="ExternalInput").ap()
    out = nc.dram_tensor("out", [T, VOCAB], F32, kind="ExternalOutput").ap()

    with tile.TileContext(nc) as tc, ExitStack() as ctx:
        dram = ctx.enter_context(tc.tile_pool(name="dram", bufs=1, space="DRAM"))
        consts = ctx.enter_context(tc.tile_pool(name="consts", bufs=1))
        ps_mm = ctx.enter_context(tc.tile_pool(name="ps_mm", bufs=2, space="PSUM"))
        ps_st = ctx.enter_context(tc.tile_pool(name="ps_st", bufs=2, space="PSUM"))
        ps_v64 = ctx.enter_context(tc.tile_pool(name="ps_v64", bufs=2, space="PSUM"))
        ps_b = ctx.enter_context(tc.tile_pool(name="ps_b", bufs=1, space="PSUM"))

        # ---- constants / small weights resident in SBUF ----
        ident = consts.tile([128, 128], F32, tag="ident")
        make_identity(nc, ident[:])
        identh = consts.tile([64, 64], MDT, tag="identh")
        nc.vector.tensor_copy(identh[:], ident[:64, :64])
        ones_f = consts.tile([128, 1], F32, tag="ones_f")
        nc.vector.memset(ones_f[:], 1.0)
        onesc = consts.tile([128, 1], MDT, tag="onesc")      # ones column
        nc.vector.tensor_copy(onesc[:], ones_f[:])
        ones_rowf = consts.tile([1, T], F32, tag="ones_rowf")
        nc.vector.memset(ones_rowf[:], 1.0)
        onesr = consts.tile([1, T], MDT, tag="onesr")        # ones row
        nc.vector.tensor_copy(onesr[:], ones_rowf[:])
        twos_f = consts.tile([1, 128], F32, tag="twos_f")
        nc.vector.memset(twos_f[:], 2.0)
        twosr = consts.tile([1, 128], MDT, tag="twosr")      # twos row
        nc.vector.tensor_copy(twosr[:], twos_f[:])
        eps_t = consts.tile([1, 1], F32, tag="eps")
        nc.vector.memset(eps_t[:], 1e-5)
        zbias = consts.tile([128, 1], F32, tag="zbias")
        nc.vector.memset(zbias[:], 0.0)
        mbias_t = consts.tile([128, GRP], F32, tag="mbias")
        nc.sync.dma_start(mbias_t[:], mbias)

        wq_t = consts.tile([128, EC * ATTN], MDT, tag="wq")
        wk_t = consts.tile([128, EC * ATTN], MDT, tag="wk")
        wv_t = consts.tile([128, EC * ATTN], MDT, tag="wv")
        for w_t, w_d in ((wq_t, wq), (wk_t, wk), (wv_t, wv)):
            nc.sync.dma_start(
                w_t.rearrange("p (ec a) -> p ec a", ec=EC),
                w_d.rearrange("(ec p) a -> p ec a", p=128))
        bqkv_t = consts.tile([ATTN, 3], F32, tag="bqkv")
        nc.sync.dma_start(bqkv_t[:], bqkv)
        wd_t = consts.tile([ATTN, EMB], MDT, tag="wd")
        nc.sync.dma_start(wd_t[:], wd)
        bd_t = consts.tile([1, EMB], MDT, tag="bd")
        nc.sync.dma_start(bd_t[:], bd)
        c1_t = consts.tile([128, HC], F32, tag="c1")
        nc.sync.dma_start(c1_t[:], c1)
        c2_t = consts.tile([1, EMB], MDT, tag="c2")
        nc.sync.dma_start(c2_t[:], c2)

        # final hidden (fp16) handed from phase 1 to the fc phase
        hfp = ctx.enter_context(tc.tile_pool(name="hfp", bufs=1))
        hfin = hfp.tile([128, EC * T], MDT, tag="hfin")

        # ================= phase 1: embed + decoder stack =================
        with ExitStack() as lctx:
            hp = lctx.enter_context(tc.tile_pool(name="hpool", bufs=1))
            lay = lctx.enter_context(tc.tile_pool(name="lay", bufs=2))
            scr = lctx.enter_context(tc.tile_pool(name="scratch", bufs=2))
            abp = lctx.enter_context(tc.tile_pool(name="abp", bufs=1))
            a1p = lctx.enter_context(tc.tile_pool(name="a1p", bufs=1))
            w1p = lctx.enter_context(tc.tile_pool(name="w1p", bufs=4))
            w2p = lctx.enter_context(tc.tile_pool(name="w2p", bufs=2))
            etp = lctx.enter_context(tc.tile_pool(name="etp", bufs=4))
            kvp = lctx.enter_context(tc.tile_pool(name="kvp", bufs=2))
            rows = lctx.enter_context(tc.tile_pool(name="rows", bufs=4))
            rows2 = lctx.enter_context(tc.tile_pool(name="rows2", bufs=2))
            up = lctx.enter_context(tc.tile_pool(name="up", bufs=3))
            embp = lctx.enter_context(tc.tile_pool(name="embp", bufs=2))

            # residual hT: [emb-part, token-free], chunk ec at cols [ec*T,(ec+1)*T)
            h_t = hp.tile([128, EC * T], F32, tag="h")

            def hcol(ec):
                return h_t[:, ec * T:(ec + 1) * T]

            # ---- embedding gather + transpose ----
            with nc.named_scope("embed"):
                for tk in range(TC):
                    idx_t = embp.tile([128, 1], I32, tag="idx")
                    nc.sync.dma_start(idx_t[:], xi[tk * 128:(tk + 1) * 128, :])
                    gat = embp.tile([128, EMB], F32, tag="gat")
                    nc.gpsimd.indirect_dma_start(
                        out=gat[:], out_offset=None, in_=emb,
                        in_offset=bass.IndirectOffsetOnAxis(ap=idx_t[:, :1], axis=0))
                    for ec in range(EC):
                        tr_ps = ps_mm.tile([128, 128], F32, tag="mm")
                        nc.tensor.transpose(
                            tr_ps[:], gat[:, ec * 128:(ec + 1) * 128], ident[:])
                        nc.vector.tensor_copy(
                            h_t[:, ec * T + tk * 128: ec * T + (tk + 1) * 128],
                            tr_ps[:])

            def layernorm(z_t):
                """z = (h - mu(h)) / sqrt(var(h)+eps), fp16 into z_t.

                istd comes from ACT Dsqrt (= 1/(2 sqrt)); the missing factor
                of 2 is folded into the twos-row broadcast matmul."""
                sum_ps = ps_st.tile([1, T], F32, tag="stat")
                sq_ps = ps_st.tile([1, T], F32, tag="stat")
                for ec in range(EC):
                    hr = scr.tile([128, T], MDT, tag="hrc")
                    nc.vector.tensor_copy(hr[:], hcol(ec))
                    hsq = scr.tile([128, T], MDT, tag="hsc")
                    nc.scalar.activation(hsq[:], hcol(ec), AF.Square)
                    nc.tensor.matmul(sum_ps[:], onesc[:], hr[:],
                                     start=(ec == 0), stop=(ec == EC - 1))
                    nc.tensor.matmul(sq_ps[:], onesc[:], hsq[:],
                                     start=(ec == 0), stop=(ec == EC - 1))
                nmu = rows.tile([1, T], F32, tag="r1")
                nc.vector.tensor_scalar(nmu[:], sum_ps[:], -1.0 / EMB, None,
                                        op0=ALU.mult)
                var = rows.tile([1, T], F32, tag="r1")
                nc.vector.tensor_scalar(var[:], sq_ps[:], 1.0 / EMB, None,
                                        op0=ALU.mult)
                musq = rows.tile([1, T], F32, tag="r1")
                nc.vector.tensor_tensor(musq[:], nmu[:], nmu[:], op=ALU.mult)
                nc.vector.tensor_tensor(var[:], var[:], musq[:], op=ALU.subtract)
                nc.vector.tensor_scalar(var[:], var[:], 1.0, 1e-5,
                                        op0=ALU.mult, op1=ALU.add)
                # rsqrt via bit-trick seed + 2 Newton steps, all on DVE
                # (keeps ACT on the exp table; no activation-table switches)
                y = rows.tile([1, T], I32, tag="r1i")
                nc.vector.tensor_scalar(y[:], var[:].bitcast(I32), 1, None,
                                        op0=ALU.logical_shift_right)
                nc.vector.tensor_scalar(y[:], y[:], -1, 0x5f3759df,
                                        op0=ALU.mult, op1=ALU.add)
                yf = y[:].bitcast(F32)
                istd = rows.tile([1, T], F32, tag="r1")
                for _ in range(2):
                    a = rows.tile([1, T], F32, tag="r1")
                    nc.vector.tensor_tensor(a[:], yf, yf, op=ALU.mult)
                    nc.vector.tensor_tensor(a[:], a[:], var[:], op=ALU.mult)
                    nc.vector.tensor_scalar(a[:], a[:], -0.5, 1.5,
                                            op0=ALU.mult, op1=ALU.add)
                    nc.vector.tensor_tensor(yf, yf, a[:], op=ALU.mult)
                nc.vector.tensor_copy(istd[:], yf)
                ab_row = rows2.tile([1, 2 * T], MDT, tag="r2")
                nc.vector.tensor_copy(ab_row[:, :T], istd[:])
                nc.vector.tensor_tensor(ab_row[:, T:], nmu[:], istd[:], op=ALU.mult)
                ab_ps = ps_b.tile([128, 2 * T], F32, tag="bcast")
                nc.tensor.matmul(ab_ps[:, :T], onesr[:, :128], ab_row[:, :T],
                                 start=True, stop=True)
                nc.tensor.matmul(ab_ps[:, T:], onesr[:, :128], ab_row[:, T:],
                                 start=True, stop=True)
                ab_sb = abp.tile([128, 2 * T], F32, tag="ab")
                nc.vector.tensor_copy(ab_sb[:], ab_ps[:])
                for ec in range(EC):
                    u = up.tile([128, T], F32, tag="u")
                    nc.vector.tensor_tensor(u[:], hcol(ec), ab_sb[:, :T],
                                            op=ALU.mult)
                    nc.vector.tensor_tensor(z_t[:, ec * T:(ec + 1) * T], u[:],
                                            ab_sb[:, T:], op=ALU.add)

            for layer in range(STACK):
                with nc.named_scope(f"L{layer}"):
                    # ---- LN1 + KV first (so the gather launches early) ----
                    z_t = scr.tile([128, EC * T], MDT, tag="scr4")
                    layernorm(z_t)
                    qkv_sb = {}
                    for name, w_t, qi in (("k", wk_t, 1), ("v", wv_t, 2),
                                          ("q", wq_t, 0)):
                        p = ps_v64.tile([ATTN, T], F32, tag="vec64")
                        for ec in range(EC):
                            nc.tensor.matmul(
                                p[:], w_t[:, ec * ATTN:(ec + 1) * ATTN],
                                z_t[:, ec * T:(ec + 1) * T],
                                start=(ec == 0), stop=(ec == EC - 1))
                        s = lay.tile([ATTN, T], MDT, tag=f"qkv{qi}")
                        nc.scalar.activation(s[:], p[:], AF.Identity,
                                             bias=bqkv_t[:, qi:qi + 1])
                        qkv_sb[name] = s
                        if name == "v":
                            # local v -> token-major, then stage k|v and gather
                            v_loc = lay.tile(
                                [128, TC * ATTN], MDT, tag="vloc")
                            qkv_sb["vloc"] = v_loc
                            for tk in range(TC):
                                tp = ps_v64.tile([128, 128], MDT, tag="vec64")
                                nc.tensor.transpose(
                                    tp[:128, :ATTN],
                                    qkv_sb["v"][:, tk * 128:(tk + 1) * 128],
                                    identh[:])
                                nc.vector.tensor_copy(
                                    v_loc[:, tk * ATTN:(tk + 1) * ATTN],
                                    tp[:128, :ATTN])
                            kv_loc = dram.tile([2 * ATTN * T], MDT, tag="kv_loc")
                            nc.sync.dma_start(
                                kv_loc[0:ATTN * T]
                                .rearrange("(a t) -> a t", a=ATTN),
                                qkv_sb["k"][:])
                            nc.sync.dma_start(
                                kv_loc[ATTN * T:].rearrange("(p c) -> p c", p=128),
                                v_loc[:])
                            kv_g = dram.tile([GRP, 2 * ATTN * T], MDT, tag="kv_g")
                            nc.gpsimd.collective_compute(
                                "AllGather", ALU.bypass, replica_groups=GROUPS,
                                ins=[kv_loc.opt()], outs=[kv_g.opt()])
                    qT = qkv_sb["q"]

                    kT = kvp.tile([ATTN, SEQ], MDT, tag="kT")
                    vtm = kvp.tile([128, KC * ATTN], MDT, tag="vtm")
                    for r in range(GRP):
                        nc.sync.dma_start(
                            kT[:, r * T:(r + 1) * T],
                            kv_g[r, 0:ATTN * T].rearrange("(a t) -> a t", a=ATTN))
                        nc.sync.dma_start(
                            vtm[:, r * TC * ATTN:(r + 1) * TC * ATTN]
                            .rearrange("p (c a) -> p c a", c=TC),
                            kv_g[r, ATTN * T:]
                            .rearrange("(p c a) -> p c a", p=128, c=TC))

                    # ---- attention ----
                    # e = exp(scoresT); AV and denominator accumulate per chunk;
                    # 1/denominator is applied to the AV product (linearity).
                    # The core's own quarter runs from local tiles while the
                    # gather is in flight; the gathered copy of that quarter is
                    # zeroed via an exp bias of -1e4 (same program on all cores,
                    # mask supplied per core).
                    den_ps = ps_st.tile([1, T], F32, tag="stat")
                    head_ps = ps_v64.tile([ATTN, T], F32, tag="vec64")
                    scale = float(ATTN) ** -0.5
                    for lk in range(TC):
                        s_ps = ps_mm.tile([128, T], F32, tag="mm")
                        nc.tensor.matmul(s_ps[:],
                                         qkv_sb["k"][:, lk * 128:(lk + 1) * 128],
                                         qT[:], start=True, stop=True)
                        e_kc = etp.tile([128, T], MDT, tag="eT")
                        nc.scalar.activation(e_kc[:], s_ps[:], AF.Exp,
                                             scale=scale, bias=zbias[:, :1])
                        nc.tensor.matmul(den_ps[:], onesc[:], e_kc[:],
                                         start=(lk == 0), stop=False)
                        nc.tensor.matmul(head_ps[:],
                                         qkv_sb["vloc"][:, lk * ATTN:(lk + 1) * ATTN],
                                         e_kc[:], start=(lk == 0), stop=False)
                    for kc in range(KC):
                        r = kc // TC
                        s_ps = ps_mm.tile([128, T], F32, tag="mm")
                        nc.tensor.matmul(s_ps[:], kT[:, kc * 128:(kc + 1) * 128],
                                         qT[:], start=True, stop=True)
                        e_kc = etp.tile([128, T], MDT, tag="eT")
                        nc.scalar.activation(e_kc[:], s_ps[:], AF.Exp,
                                             scale=scale, bias=mbias_t[:, r:r + 1])
                        nc.tensor.matmul(den_ps[:], onesc[:], e_kc[:],
                                         start=False, stop=(kc == KC - 1))
                        nc.tensor.matmul(head_ps[:],
                                         vtm[:, kc * ATTN:(kc + 1) * ATTN],
                                         e_kc[:],
                                         start=False, stop=(kc == KC - 1))
                    recip = rows.tile([1, T], F32, tag="r1")
                    nc.vector.reciprocal(recip[:], den_ps[:])
                    rrow = rows.tile([1, T], MDT, tag="r1")
                    nc.vector.tensor_copy(rrow[:], recip[:])
                    rb_ps = ps_b.tile([128, 2 * T], F32, tag="bcast")
                    nc.tensor.matmul(rb_ps[:ATTN, :T], onesr[:, :ATTN], rrow[:],
                                     start=True, stop=True)
                    rb_sb = abp.tile([ATTN, T], F32, tag="rb")
                    nc.vector.tensor_copy(rb_sb[:], rb_ps[:ATTN, :T])
                    headT = lay.tile([ATTN, T], MDT, tag="headT")
                    nc.vector.tensor_tensor(headT[:], head_ps[:], rb_sb[:],
                                            op=ALU.mult)

                    # ---- proj + residual ----
                    for ec in range(EC):
                        p_ps = ps_mm.tile([128, T], F32, tag="mm")
                        nc.tensor.matmul(p_ps[:], wd_t[:, ec * 128:(ec + 1) * 128],
                                         headT[:], start=True, stop=False)
                        nc.tensor.matmul(p_ps[:], bd_t[:, ec * 128:(ec + 1) * 128],
                                         onesr[:], start=False, stop=True)
                        nc.vector.tensor_tensor(hcol(ec), hcol(ec), p_ps[:],
                                                op=ALU.add)

                    # ---- LN2 + FFN (two half passes over hidden chunks) ----
                    z2_t = scr.tile([128, EC * T], MDT, tag="scr4")
                    layernorm(z2_t)
                    for half in range(2):
                        a1 = a1p.tile([128, (HC // 2) * T], MDT, tag="a1")
                        for j in range(HC // 2):
                            hc = half * (HC // 2) + j
                            w1_t = w1p.tile([128, EC * 128], MDT, tag="w1")
                            nc.sync.dma_start(w1_t[:], w1[hc])
                            f_ps = ps_mm.tile([128, T], F32, tag="mm")
                            for ec in range(EC):
                                nc.tensor.matmul(
                                    f_ps[:], w1_t[:, ec * 128:(ec + 1) * 128],
                                    z2_t[:, ec * T:(ec + 1) * T],
                                    start=(ec == 0), stop=(ec == EC - 1))
                            nc.scalar.activation(a1[:, j * T:(j + 1) * T], f_ps[:],
                                                 AF.Relu, bias=c1_t[:, hc:hc + 1])
                        for ec in range(EC):
                            w2_t = w2p.tile([128, (HC // 2) * 128], MDT, tag="w2")
                            nc.sync.dma_start(w2_t[:], w2[half, ec])
                            g_ps = ps_mm.tile([128, T], F32, tag="mm")
                            for j in range(HC // 2):
                                nc.tensor.matmul(
                                    g_ps[:], w2_t[:, j * 128:(j + 1) * 128],
                                    a1[:, j * T:(j + 1) * T],
                                    start=(j == 0),
                                    stop=(j == HC // 2 - 1 and half == 1))
                            if half == 1:
                                nc.tensor.matmul(
                                    g_ps[:], c2_t[:, ec * 128:(ec + 1) * 128],
                                    onesr[:], start=False, stop=True)
                            nc.vector.tensor_tensor(hcol(ec), hcol(ec), g_ps[:],
                                                    op=ALU.add)

            # ---- final hidden to fp16 for the local-token fc ----
            with nc.named_scope("hfin"):
                nc.vector.tensor_copy(hfin[:], h_t[:])

        # ======= phase 2: fc, local tokens x full vocab (no collective) =======
        with nc.named_scope("fc"):
            with tc.tile_pool(name="wfcp", bufs=3) as wfcp, \
                 tc.tile_pool(name="outp", bufs=4) as outp, \
                 tc.tile_pool(name="bfcp", bufs=2) as bfcp:
                NVC = VOCAB // VCW
                for vc in range(NVC):
                    wfc_t = wfcp.tile([128, EC * VCW], MDT, tag="wfc")
                    nc.sync.dma_start(wfc_t[:], wfc[vc])
                    bfc_t = bfcp.tile([1, VCW], MDT, tag="bfc")
                    nc.sync.dma_start(bfc_t[:], bfc[vc:vc + 1, :])
                    for tcg in range(TC):
                        o_ps = ps_mm.tile([128, VCW], F32, tag="mm")
                        for ec in range(EC):
                            nc.tensor.matmul(
                                o_ps[:],
                                hfin[:, ec * T + tcg * 128:
                                     ec * T + (tcg + 1) * 128],
                                wfc_t[:, ec * VCW:(ec + 1) * VCW],
                                start=(ec == 0), stop=False)
                        nc.tensor.matmul(o_ps[:], onesr[:, :128], bfc_t[:],
                                         start=False, stop=True)
                        o_sb = outp.tile([128, VCW], F32, tag="osb")
                        nc.vector.tensor_copy(o_sb[:], o_ps[:])
                        nc.sync.dma_start(
                            out[tcg * 128:(tcg + 1) * 128,
                                vc * VCW:(vc + 1) * VCW], o_sb[:])

    nc.compile()
    return nc


_NC_CACHE = None


def _get_nc():
    global _NC_CACHE
    if _NC_CACHE is None:
        _NC_CACHE = build_nc()
    return _NC_CACHE


def prepare_in_maps(inputs):
    f32 = np.float32
    x = np.asarray(inputs["x"]).reshape(-1).astype(np.int32)
    emb = np.ascontiguousarray(np.asarray(inputs["emb"], f32))
    g1 = np.asarray(inputs["g1"], f32)
    beta1 = np.asarray(inputs["beta1"], f32)
    g2 = np.asarray(inputs["g2"], f32)
    beta2 = np.asarray(inputs["beta2"], f32)
    Wq = np.asarray(inputs["Wq"], f32)
    Wk = np.asarray(inputs["Wk"], f32)
    Wv = np.asarray(inputs["Wv"], f32)
    # fold LN1 affine into qkv projections
    wq_f = np.ascontiguousarray((g1[:, None] * Wq).astype(NDT))
    wk_f = np.ascontiguousarray((g1[:, None] * Wk).astype(NDT))
    wv_f = np.ascontiguousarray((g1[:, None] * Wv).astype(NDT))
    bq_f = np.asarray(inputs["bq"], f32) + beta1 @ Wq
    bk_f = np.asarray(inputs["bk"], f32) + beta1 @ Wk
    bv_f = np.asarray(inputs["bv"], f32) + beta1 @ Wv
    bqkv = np.ascontiguousarray(np.stack([bq_f, bk_f, bv_f], axis=1))  # [64,3]
    # tile(head, 16) @ Wd == head @ (sum of the 16 row-blocks of Wd)
    Wd_sum = np.asarray(inputs["Wd"], f32).reshape(N_HEADS, ATTN, EMB).sum(0)
    wd_h = np.ascontiguousarray(Wd_sum.astype(NDT))
    bd = np.ascontiguousarray(np.asarray(inputs["bd"], f32)[None, :].astype(NDT))
    # fold LN2 affine into W1; swizzle to [hc][p][ec*128]
    W1 = np.asarray(inputs["W1"], f32)
    w1_f = (g2[:, None] * W1).astype(NDT)                    # [1024, 4096]
    w1_sw = np.ascontiguousarray(
        w1_f.reshape(EC, 128, HC, 128).transpose(2, 1, 0, 3)
        .reshape(HC, 128, EC * 128))
    c1_f = np.asarray(inputs["c1"], f32) + beta2 @ W1
    c1_t = np.ascontiguousarray(c1_f.reshape(HC, 128).T)     # [128, HC]
    # W2 swizzle to [half][ec][p][j*128]
    W2 = np.asarray(inputs["W2"], f32).astype(NDT)           # [4096, 1024]
    w2_sw = np.ascontiguousarray(
        W2.reshape(2, HC // 2, 128, EC, 128).transpose(0, 3, 2, 1, 4)
        .reshape(2, EC, 128, (HC // 2) * 128))
    c2 = np.ascontiguousarray(np.asarray(inputs["c2"], f32)[None, :].astype(NDT))
    Wfc = np.asarray(inputs["Wfc"], f32)
    bfc = np.asarray(inputs["bfc"], f32)

    NVC = VOCAB // VCW
    wfc_sw = np.ascontiguousarray(
        Wfc.astype(NDT).reshape(EC, 128, NVC, VCW).transpose(2, 1, 0, 3)
        .reshape(NVC, 128, EC * VCW))
    bfc_sw = np.ascontiguousarray(bfc.astype(NDT).reshape(NVC, VCW))
    in_maps = []
    for c in range(N_CORES):
        mb = np.zeros((128, GRP), np.float32)
        mb[:, c % GRP] = -1e4
        in_maps.append(dict(
            emb=emb,
            xi=np.ascontiguousarray(x[c * T:(c + 1) * T, None]),
            wq=wq_f, wk=wk_f, wv=wv_f, bqkv=bqkv,
            wd=wd_h, bd=bd, w1=w1_sw, c1=c1_t, w2=w2_sw, c2=c2,
            wfc=wfc_sw, bfc=bfc_sw, mbias=mb,
        ))
    return in_maps


def kernel(**inputs) -> np.ndarray:
    nc = _get_nc()
    in_maps = prepare_in_maps(inputs)
    r = run_bass_kernel_spmd(nc, in_maps, core_ids=list(range(N_CORES)))
    logits = np.concatenate([r.results[c]["out"] for c in range(N_CORES)], axis=0)
    return logits.reshape(BATCH, SEQ, VOCAB)


# revision 16
# speedup vs baseline: 1.2661x; 1.0076x over previous
"""Trainium2 Bass kernel for an 8-layer weight-shared decoder stack (v2, fp16).

Model (see problem reference): h = emb[x]; 8x identical decoder layers
(LN -> single-head attn tiled 16x -> proj -> LN -> 4x FFN); fc to vocab.

Distribution over 8 NeuronCores:
  - tokens sharded 8-way (cores 0-3 <- batch 0, cores 4-7 <- batch 1;
    512 tokens per core); per-layer AllGather of K/V within each 4-core
    batch group;
  - final hidden states AllGathered across all 8 cores; fc vocab-sharded
    (4000 columns per core); host concatenates the vocab shards.

Numerics: fp16 matmul operands (11-bit mantissa, same error class as
fp32r but with hideable LDWEIGHTS and FWL), fp32 residual stream and
fp32 PSUM accumulation everywhere.
Algebraic folds: tile(head,16) @ Wd == head @ Wd_sum; LN affine (g, beta)
folded into the following weight matrices; softmax denominator applied
to the AV product instead of the probabilities (linearity).
Activations are stored transposed (embedding on partitions) so no
activation transposes are needed anywhere; attention scores are computed
directly in [key, query] layout and the softmax reductions over keys run
on the PE via ones-vector matmuls.
Large weights (W1/W2/Wfc) are passed pre-swizzled so every tile load is
one contiguous run per partition (no DMA descriptor fragmentation).
"""
import numpy as np
from contextlib import ExitStack

import concourse.bass as bass
import concourse.tile as tile
from concourse import bacc, mybir
from concourse.bass_utils import run_bass_kernel_spmd
from concourse.masks import make_identity

dt = mybir.dt
AF = mybir.ActivationFunctionType
ALU = mybir.AluOpType

# model dims (hardcoded per the problem spec)
VOCAB, EMB, SEQ, STACK, N_HEADS, ATTN, BATCH = 32000, 1024, 2048, 8, 16, 64, 2
N_CORES = 8
T = (BATCH * SEQ) // N_CORES          # 512 tokens per core
GRP = 4                               # cores per batch group
GROUPS = [[0, 1, 2, 3], [4, 5, 6, 7]]
EC = EMB // 128                       # 8 emb chunks
KC = SEQ // 128                       # 16 key chunks (per batch)
HC = 4 * EMB // 128                   # 32 ffn hidden chunks
TC = T // 128                         # 4 local token chunks
VSH = VOCAB // N_CORES                # 4000 vocab per core
VCC = 8                               # vocab col chunks per core
VCW = VSH // VCC                      # 500 cols per chunk
GTC = (BATCH * SEQ) // 128            # 32 global token chunks
F32, I32 = dt.float32, dt.int32
MDT = dt.float16                      # matmul operand dtype
NDT = np.float16


def build_nc():
    nc = bacc.Bacc("TRN2", target_bir_lowering=False, debug=False,
                   enable_asserts=True, num_devices=N_CORES)

    # ---- I/O ----  (w1/w2/wfc are host-swizzled; see prepare_in_maps)
    emb = nc.dram_tensor("emb", [VOCAB, EMB], F32, kind="ExternalInput").ap()
    xi = nc.dram_tensor("xi", [T, 1], I32, kind="ExternalInput").ap()
    wq = nc.dram_tensor("wq", [EMB, ATTN], MDT, kind="ExternalInput").ap()
    wk = nc.dram_tensor("wk", [EMB, ATTN], MDT, kind="ExternalInput").ap()
    wv = nc.dram_tensor("wv", [EMB, ATTN], MDT, kind="ExternalInput").ap()
    bqkv = nc.dram_tensor("bqkv", [ATTN, 3], F32, kind="ExternalInput").ap()
    wd = nc.dram_tensor("wd", [ATTN, EMB], MDT, kind="ExternalInput").ap()  # Wd_sum
    bd = nc.dram_tensor("bd", [1, EMB], MDT, kind="ExternalInput").ap()
    w1 = nc.dram_tensor("w1", [HC, 128, EC * 128], MDT,
                        kind="ExternalInput").ap()          # [hc][p][ec*m]
    c1 = nc.dram_tensor("c1", [128, HC], F32, kind="ExternalInput").ap()
    w2 = nc.dram_tensor("w2", [2, EC, 128, (HC // 2) * 128], MDT,
                        kind="ExternalInput").ap()          # [half][ec][p][j*m]
    c2 = nc.dram_tensor("c2", [1, EMB], MDT, kind="ExternalInput").ap()
    wfc = nc.dram_tensor("wfc", [VOCAB // VCW, 128, EC * VCW], MDT,
                         kind="ExternalInput").ap()         # [vc][p][ec*n]
    bfc = nc.dram_tensor("bfc", [VOCAB // VCW, VCW], MDT, kind="ExternalInput").ap()
    mbias = nc.dram_tensor("mbias", [128, GRP], F32, kind="ExternalInput").ap()
    out = nc.dram_tensor("out", [T, VOCAB], F32, kind="ExternalOutput").ap()

    with tile.TileContext(nc) as tc, ExitStack() as ctx:
        dram = ctx.enter_context(tc.tile_pool(name="dram", bufs=1, space="DRAM"))
        consts = ctx.enter_context(tc.tile_pool(name="consts", bufs=1))
        ps_mm = ctx.enter_context(tc.tile_pool(name="ps_mm", bufs=3, space="PSUM"))
        ps_st = ctx.enter_context(tc.tile_pool(name="ps_st", bufs=2, space="PSUM"))
        ps_v64 = ctx.enter_context(tc.tile_pool(name="ps_v64", bufs=2, space="PSUM"))
        ps_b = ctx.enter_context(tc.tile_pool(name="ps_b", bufs=1, space="PSUM"))

        # ---- constants / small weights resident in SBUF ----
        ident = consts.tile([128, 128], F32, tag="ident")
        make_identity(nc, ident[:])
        identh = consts.tile([64, 64], MDT, tag="identh")
        nc.vector.tensor_copy(identh[:], ident[:64, :64])
        ones_f = consts.tile([128, 1], F32, tag="ones_f")
        nc.vector.memset(ones_f[:], 1.0)
        onesc = consts.tile([128, 1], MDT, tag="onesc")      # ones column
        nc.vector.tensor_copy(onesc[:], ones_f[:])
        ones_rowf = consts.tile([1, T], F32, tag="ones_rowf")
        nc.vector.memset(ones_rowf[:], 1.0)
        onesr = consts.tile([1, T], MDT, tag="onesr")        # ones row
        nc.vector.tensor_copy(onesr[:], ones_rowf[:])
        twos_f = consts.tile([1, 128], F32, tag="twos_f")
        nc.vector.memset(twos_f[:], 2.0)
        twosr = consts.tile([1, 128], MDT, tag="twosr")      # twos row
        nc.vector.tensor_copy(twosr[:], twos_f[:])
        eps_t = consts.tile([1, 1], F32, tag="eps")
        nc.vector.memset(eps_t[:], 1e-5)
        zbias = consts.tile([128, 1], F32, tag="zbias")
        nc.vector.memset(zbias[:], 0.0)
        mbias_t = consts.tile([128, GRP], F32, tag="mbias")
        nc.sync.dma_start(mbias_t[:], mbias)

        wq_t = consts.tile([128, EC * ATTN], MDT, tag="wq")
        wk_t = consts.tile([128, EC * ATTN], MDT, tag="wk")
        wv_t = consts.tile([128, EC * ATTN], MDT, tag="wv")
        for w_t, w_d in ((wq_t, wq), (wk_t, wk), (wv_t, wv)):
            nc.sync.dma_start(
                w_t.rearrange("p (ec a) -> p ec a", ec=EC),
                w_d.rearrange("(ec p) a -> p ec a", p=128))
        bqkv_t = consts.tile([ATTN, 3], F32, tag="bqkv")
        nc.sync.dma_start(bqkv_t[:], bqkv)
        wd_t = consts.tile([ATTN, EMB], MDT, tag="wd")
        nc.sync.dma_start(wd_t[:], wd)
        bd_t = consts.tile([1, EMB], MDT, tag="bd")
        nc.sync.dma_start(bd_t[:], bd)
        c1_t = consts.tile([128, HC], F32, tag="c1")
        nc.sync.dma_start(c1_t[:], c1)
        c2_t = consts.tile([1, EMB], MDT, tag="c2")
        nc.sync.dma_start(c2_t[:], c2)

        # final hidden (fp16) handed from phase 1 to the fc phase
        hfp = ctx.enter_context(tc.tile_pool(name="hfp", bufs=1))
        hfin = hfp.tile([128, EC * T], MDT, tag="hfin")

        # ================= phase 1: embed + decoder stack =================
        with ExitStack() as lctx:
            hp = lctx.enter_context(tc.tile_pool(name="hpool", bufs=1))
            lay = lctx.enter_context(tc.tile_pool(name="lay", bufs=2))
            scr = lctx.enter_context(tc.tile_pool(name="scratch", bufs=2))
            abp = lctx.enter_context(tc.tile_pool(name="abp", bufs=1))
            a1p = lctx.enter_context(tc.tile_pool(name="a1p", bufs=1))
            w1p = lctx.enter_context(tc.tile_pool(name="w1p", bufs=4))
            w2p = lctx.enter_context(tc.tile_pool(name="w2p", bufs=2))
            etp = lctx.enter_context(tc.tile_pool(name="etp", bufs=4))
            kvp = lctx.enter_context(tc.tile_pool(name="kvp", bufs=2))
            rows = lctx.enter_context(tc.tile_pool(name="rows", bufs=4))
            rows2 = lctx.enter_context(tc.tile_pool(name="rows2", bufs=2))
            up = lctx.enter_context(tc.tile_pool(name="up", bufs=3))
            embp = lctx.enter_context(tc.tile_pool(name="embp", bufs=2))

            # residual hT: [emb-part, token-free], chunk ec at cols [ec*T,(ec+1)*T)
            h_t = hp.tile([128, EC * T], F32, tag="h")

            def hcol(ec):
                return h_t[:, ec * T:(ec + 1) * T]

            # ---- embedding gather + transpose ----
            with nc.named_scope("embed"):
                for tk in range(TC):
                    idx_t = embp.tile([128, 1], I32, tag="idx")
                    nc.sync.dma_start(idx_t[:], xi[tk * 128:(tk + 1) * 128, :])
                    gat = embp.tile([128, EMB], F32, tag="gat")
                    nc.gpsimd.indirect_dma_start(
                        out=gat[:], out_offset=None, in_=emb,
                        in_offset=bass.IndirectOffsetOnAxis(ap=idx_t[:, :1], axis=0))
                    for ec in range(EC):
                        tr_ps = ps_mm.tile([128, 128], F32, tag="mm")
                        nc.tensor.transpose(
                            tr_ps[:], gat[:, ec * 128:(ec + 1) * 128], ident[:])
                        nc.vector.tensor_copy(
                            h_t[:, ec * T + tk * 128: ec * T + (tk + 1) * 128],
                            tr_ps[:])

            def layernorm(z_t):
                """z = (h - mu(h)) / sqrt(var(h)+eps), fp16 into z_t.

                istd comes from ACT Dsqrt (= 1/(2 sqrt)); the missing factor
                of 2 is folded into the twos-row broadcast matmul."""
                sum_ps = ps_st.tile([1, T], F32, tag="stat")
                sq_ps = ps_st.tile([1, T], F32, tag="stat")
                for ec in range(EC):
                    hr = scr.tile([128, T], MDT, tag="hrc")
                    nc.vector.tensor_copy(hr[:], hcol(ec))
                    hsq = scr.tile([128, T], MDT, tag="hsc")
                    nc.scalar.activation(hsq[:], hcol(ec), AF.Square)
                    nc.tensor.matmul(sum_ps[:], onesc[:], hr[:],
                                     start=(ec == 0), stop=(ec == EC - 1))
                    nc.tensor.matmul(sq_ps[:], onesc[:], hsq[:],
                                     start=(ec == 0), stop=(ec == EC - 1))
                nmu = rows.tile([1, T], F32, tag="r1")
                nc.vector.tensor_scalar(nmu[:], sum_ps[:], -1.0 / EMB, None,
                                        op0=ALU.mult)
                var = rows.tile([1, T], F32, tag="r1")
                nc.vector.tensor_scalar(var[:], sq_ps[:], 1.0 / EMB, None,
                                        op0=ALU.mult)
                musq = rows.tile([1, T], F32, tag="r1")
                nc.vector.tensor_tensor(musq[:], nmu[:], nmu[:], op=ALU.mult)
                nc.vector.tensor_tensor(var[:], var[:], musq[:], op=ALU.subtract)
                nc.vector.tensor_scalar(var[:], var[:], 1.0, 1e-5,
                                        op0=ALU.mult, op1=ALU.add)
                # rsqrt via bit-trick seed + 2 Newton steps, all on DVE
                # (keeps ACT on the exp table; no activation-table switches)
                y = rows.tile([1, T], I32, tag="r1i")
                nc.vector.tensor_scalar(y[:], var[:].bitcast(I32), 1, None,
                                        op0=ALU.logical_shift_right)
                nc.vector.tensor_scalar(y[:], y[:], -1, 0x5f3759df,
                                        op0=ALU.mult, op1=ALU.add)
                yf = y[:].bitcast(F32)
                istd = rows.tile([1, T], F32, tag="r1")
                for _ in range(2):
                    a = rows.tile([1, T], F32, tag="r1")
                    nc.vector.tensor_tensor(a[:], yf, yf, op=ALU.mult)
                    nc.vector.tensor_tensor(a[:], a[:], var[:], op=ALU.mult)
                    nc.vector.tensor_scalar(a[:], a[:], -0.5, 1.5,
                                            op0=ALU.mult, op1=ALU.add)
                    nc.vector.tensor_tensor(yf, yf, a[:], op=ALU.mult)
                nc.vector.tensor_copy(istd[:], yf)
                ab_row = rows2.tile([1, 2 * T], MDT, tag="r2")
                nc.vector.tensor_copy(ab_row[:, :T], istd[:])
                nc.vector.tensor_tensor(ab_row[:, T:], nmu[:], istd[:], op=ALU.mult)
                ab_sb = abp.tile([128, 2 * T], F32, tag="ab")
                a_ps = ps_b.tile([128, T], F32, tag="bcast")
                nc.tensor.matmul(a_ps[:], onesr[:, :128], ab_row[:, :T],
                                 start=True, stop=True)
                nc.vector.tensor_copy(ab_sb[:, :T], a_ps[:])
                b_ps = ps_b.tile([128, T], F32, tag="bcast")
                nc.tensor.matmul(b_ps[:], onesr[:, :128], ab_row[:, T:],
                                 start=True, stop=True)
                nc.vector.tensor_copy(ab_sb[:, T:], b_ps[:])
                for ec in range(EC):
                    u = up.tile([128, T], F32, tag="u")
                    nc.vector.tensor_tensor(u[:], hcol(ec), ab_sb[:, :T],
                                            op=ALU.mult)
                    nc.vector.tensor_tensor(z_t[:, ec * T:(ec + 1) * T], u[:],
                                            ab_sb[:, T:], op=ALU.add)

            for layer in range(STACK):
                with nc.named_scope(f"L{layer}"):
                    # ---- LN1 + KV first (so the gather launches early) ----
                    z_t = scr.tile([128, EC * T], MDT, tag="scr4")
                    layernorm(z_t)
                    qkv_sb = {}
                    for name, w_t, qi in (("k", wk_t, 1), ("v", wv_t, 2),
                                          ("q", wq_t, 0)):
                        p = ps_v64.tile([ATTN, T], F32, tag="vec64")
                        for ec in range(EC):
                            nc.tensor.matmul(
                                p[:], w_t[:, ec * ATTN:(ec + 1) * ATTN],
                                z_t[:, ec * T:(ec + 1) * T],
                                start=(ec == 0), stop=(ec == EC - 1))
                        s = lay.tile([ATTN, T], MDT, tag=f"qkv{qi}")
                        nc.scalar.activation(s[:], p[:], AF.Identity,
                                             bias=bqkv_t[:, qi:qi + 1])
                        qkv_sb[name] = s
                        if name == "v":
                            # local v -> token-major, then stage k|v and gather
                            v_loc = lay.tile(
                                [128, TC * ATTN], MDT, tag="vloc")
                            qkv_sb["vloc"] = v_loc
                            for tk in range(TC):
                                tp = ps_v64.tile([128, 128], MDT, tag="vec64")
                                nc.tensor.transpose(
                                    tp[:128, :ATTN],
                                    qkv_sb["v"][:, tk * 128:(tk + 1) * 128],
                                    identh[:])
                                nc.vector.tensor_copy(
                                    v_loc[:, tk * ATTN:(tk + 1) * ATTN],
                                    tp[:128, :ATTN])
                            kv_loc = dram.tile([2 * ATTN * T], MDT, tag="kv_loc")
                            nc.sync.dma_start(
                                kv_loc[0:ATTN * T]
                                .rearrange("(a t) -> a t", a=ATTN),
                                qkv_sb["k"][:])
                            nc.sync.dma_start(
                                kv_loc[ATTN * T:].rearrange("(p c) -> p c", p=128),
                                v_loc[:])
                            kv_g = dram.tile([GRP, 2 * ATTN * T], MDT, tag="kv_g")
                            nc.gpsimd.collective_compute(
                                "AllGather", ALU.bypass, replica_groups=GROUPS,
                                ins=[kv_loc.opt()], outs=[kv_g.opt()])
                    qT = qkv_sb["q"]

                    kT = kvp.tile([ATTN, SEQ], MDT, tag="kT")
                    vtm = kvp.tile([128, KC * ATTN], MDT, tag="vtm")
                    for r in range(GRP):
                        nc.sync.dma_start(
                            kT[:, r * T:(r + 1) * T],
                            kv_g[r, 0:ATTN * T].rearrange("(a t) -> a t", a=ATTN))
                        nc.sync.dma_start(
                            vtm[:, r * TC * ATTN:(r + 1) * TC * ATTN]
                            .rearrange("p (c a) -> p c a", c=TC),
                            kv_g[r, ATTN * T:]
                            .rearrange("(p c a) -> p c a", p=128, c=TC))

                    # ---- attention ----
                    # e = exp(scoresT); AV and denominator accumulate per chunk;
                    # 1/denominator is applied to the AV product (linearity).
                    # The core's own quarter runs from local tiles while the
                    # gather is in flight; the gathered copy of that quarter is
                    # zeroed via an exp bias of -1e4 (same program on all cores,
                    # mask supplied per core).
                    den_ps = ps_st.tile([1, T], F32, tag="stat")
                    head_ps = ps_v64.tile([ATTN, T], F32, tag="vec64")
                    scale = float(ATTN) ** -0.5
                    for lk in range(TC):
                        s_ps = ps_mm.tile([128, T], F32, tag="mm")
                        nc.tensor.matmul(s_ps[:],
                                         qkv_sb["k"][:, lk * 128:(lk + 1) * 128],
                                         qT[:], start=True, stop=True)
                        e_kc = etp.tile([128, T], MDT, tag="eT")
                        nc.scalar.activation(e_kc[:], s_ps[:], AF.Exp,
                                             scale=scale, bias=zbias[:, :1])
                        nc.tensor.matmul(den_ps[:], onesc[:], e_kc[:],
                                         start=(lk == 0), stop=False)
                        nc.tensor.matmul(head_ps[:],
                                         qkv_sb["vloc"][:, lk * ATTN:(lk + 1) * ATTN],
                                         e_kc[:], start=(lk == 0), stop=False)
                    for kc in range(KC):
                        r = kc // TC
                        s_ps = ps_mm.tile([128, T], F32, tag="mm")
                        nc.tensor.matmul(s_ps[:], kT[:, kc * 128:(kc + 1) * 128],
                                         qT[:], start=True, stop=True)
                        e_kc = etp.tile([128, T], MDT, tag="eT")
                        nc.scalar.activation(e_kc[:], s_ps[:], AF.Exp,
                                             scale=scale, bias=mbias_t[:, r:r + 1])
                        nc.tensor.matmul(den_ps[:], onesc[:], e_kc[:],
                                         start=False, stop=(kc == KC - 1))
                        nc.tensor.matmul(head_ps[:],
                                         vtm[:, kc * ATTN:(kc + 1) * ATTN],
                                         e_kc[:],
                                         start=False, stop=(kc == KC - 1))
                    # reciprocal via bit-trick seed + 2 Newton steps (DVE)
                    den_sb = rows.tile([1, T], F32, tag="r1")
                    nc.vector.tensor_copy(den_sb[:], den_ps[:])
                    ry = rows.tile([1, T], I32, tag="r1i")
                    nc.vector.tensor_scalar(ry[:], den_sb[:].bitcast(I32), -1,
                                            0x7EF311C3, op0=ALU.mult, op1=ALU.add)
                    ryf = ry[:].bitcast(F32)
                    for _ in range(2):
                        ra = rows.tile([1, T], F32, tag="r1")
                        nc.vector.tensor_tensor(ra[:], ryf, den_sb[:], op=ALU.mult)
                        nc.vector.tensor_scalar(ra[:], ra[:], -1.0, 2.0,
                                                op0=ALU.mult, op1=ALU.add)
                        nc.vector.tensor_tensor(ryf, ryf, ra[:], op=ALU.mult)
                    rrow = rows.tile([1, T], MDT, tag="r1")
                    nc.vector.tensor_copy(rrow[:], ryf)
                    rb_ps = ps_b.tile([128, T], F32, tag="bcast")
                    nc.tensor.matmul(rb_ps[:ATTN, :], onesr[:, :ATTN], rrow[:],
                                     start=True, stop=True)
                    rb_sb = abp.tile([ATTN, T], F32, tag="rb")
                    nc.vector.tensor_copy(rb_sb[:], rb_ps[:ATTN, :])
                    headT = lay.tile([ATTN, T], MDT, tag="headT")
                    nc.vector.tensor_tensor(headT[:], head_ps[:], rb_sb[:],
                                            op=ALU.mult)

                    # ---- proj + residual ----
                    for ec in range(EC):
                        p_ps = ps_mm.tile([128, T], F32, tag="mm")
                        nc.tensor.matmul(p_ps[:], bd_t[:, ec * 128:(ec + 1) * 128],
                                         onesr[:], start=True, stop=False)
                        nc.tensor.matmul(p_ps[:], wd_t[:, ec * 128:(ec + 1) * 128],
                                         headT[:], start=False, stop=True)
                        nc.vector.tensor_tensor(hcol(ec), hcol(ec), p_ps[:],
                                                op=ALU.add)

                    # ---- LN2 + FFN (two half passes over hidden chunks) ----
                    z2_t = scr.tile([128, EC * T], MDT, tag="scr4")
                    layernorm(z2_t)
                    for half in range(2):
                        a1 = a1p.tile([128, (HC // 2) * T], MDT, tag="a1")
                        for j in range(HC // 2):
                            hc = half * (HC // 2) + j
                            w1_t = w1p.tile([128, EC * 128], MDT, tag="w1")
                            nc.sync.dma_start(w1_t[:], w1[hc])
                            f_ps = ps_mm.tile([128, T], F32, tag="mm")
                            for ec in range(EC):
                                nc.tensor.matmul(
                                    f_ps[:], w1_t[:, ec * 128:(ec + 1) * 128],
                                    z2_t[:, ec * T:(ec + 1) * T],
                                    start=(ec == 0), stop=(ec == EC - 1))
                            nc.scalar.activation(a1[:, j * T:(j + 1) * T], f_ps[:],
                                                 AF.Relu, bias=c1_t[:, hc:hc + 1])
                        for ec in range(EC):
                            w2_t = w2p.tile([128, (HC // 2) * 128], MDT, tag="w2")
                            nc.sync.dma_start(w2_t[:], w2[half, ec])
                            g_ps = ps_mm.tile([128, T], F32, tag="mm")
                            if half == 1:
                                nc.tensor.matmul(
                                    g_ps[:], c2_t[:, ec * 128:(ec + 1) * 128],
                                    onesr[:], start=True, stop=False)
                            for j in range(HC // 2):
                                nc.tensor.matmul(
                                    g_ps[:], w2_t[:, j * 128:(j + 1) * 128],
                                    a1[:, j * T:(j + 1) * T],
                                    start=(j == 0 and half == 0),
                                    stop=(j == HC // 2 - 1))
                            nc.vector.tensor_tensor(hcol(ec), hcol(ec), g_ps[:],
                                                    op=ALU.add)

            # ---- final hidden to fp16 for the local-token fc ----
            with nc.named_scope("hfin"):
                nc.vector.tensor_copy(hfin[:], h_t[:])

        # ======= phase 2: fc, local tokens x full vocab (no collective) =======
        with nc.named_scope("fc"):
            with tc.tile_pool(name="wfcp", bufs=3) as wfcp, \
                 tc.tile_pool(name="outp", bufs=4) as outp, \
                 tc.tile_pool(name="bfcp", bufs=2) as bfcp:
                NVC = VOCAB // VCW
                for vc in range(NVC):
                    wfc_t = wfcp.tile([128, EC * VCW], MDT, tag="wfc")
                    nc.sync.dma_start(wfc_t[:], wfc[vc])
                    bfc_t = bfcp.tile([1, VCW], MDT, tag="bfc")
                    nc.sync.dma_start(bfc_t[:], bfc[vc:vc + 1, :])
                    for tcg in range(TC):
                        o_ps = ps_mm.tile([128, VCW], F32, tag="mm")
                        for ec in range(EC):
                            nc.tensor.matmul(
                                o_ps[:],
                                hfin[:, ec * T + tcg * 128:
                                     ec * T + (tcg + 1) * 128],
                                wfc_t[:, ec * VCW:(ec + 1) * VCW],
                                start=(ec == 0), stop=False)
                        nc.tensor.matmul(o_ps[:], onesr[:, :128], bfc_t[:],
                                         start=False, stop=True)
                        o_sb = outp.tile([128, VCW], F32, tag="osb")
                        nc.vector.tensor_copy(o_sb[:], o_ps[:])
                        nc.sync.dma_start(
                            out[tcg * 128:(tcg + 1) * 128,
                                vc * VCW:(vc + 1) * VCW], o_sb[:])

    nc.compile()
    return nc


_NC_CACHE = None


def _get_nc():
    global _NC_CACHE
    if _NC_CACHE is None:
        _NC_CACHE = build_nc()
    return _NC_CACHE


def prepare_in_maps(inputs):
    f32 = np.float32
    x = np.asarray(inputs["x"]).reshape(-1).astype(np.int32)
    emb = np.ascontiguousarray(np.asarray(inputs["emb"], f32))
    g1 = np.asarray(inputs["g1"], f32)
    beta1 = np.asarray(inputs["beta1"], f32)
    g2 = np.asarray(inputs["g2"], f32)
    beta2 = np.asarray(inputs["beta2"], f32)
    Wq = np.asarray(inputs["Wq"], f32)
    Wk = np.asarray(inputs["Wk"], f32)
    Wv = np.asarray(inputs["Wv"], f32)
    # fold LN1 affine into qkv projections
    wq_f = np.ascontiguousarray((g1[:, None] * Wq).astype(NDT))
    wk_f = np.ascontiguousarray((g1[:, None] * Wk).astype(NDT))
    wv_f = np.ascontiguousarray((g1[:, None] * Wv).astype(NDT))
    bq_f = np.asarray(inputs["bq"], f32) + beta1 @ Wq
    bk_f = np.asarray(inputs["bk"], f32) + beta1 @ Wk
    bv_f = np.asarray(inputs["bv"], f32) + beta1 @ Wv
    bqkv = np.ascontiguousarray(np.stack([bq_f, bk_f, bv_f], axis=1))  # [64,3]
    # tile(head, 16) @ Wd == head @ (sum of the 16 row-blocks of Wd)
    Wd_sum = np.asarray(inputs["Wd"], f32).reshape(N_HEADS, ATTN, EMB).sum(0)
    wd_h = np.ascontiguousarray(Wd_sum.astype(NDT))
    bd = np.ascontiguousarray(np.asarray(inputs["bd"], f32)[None, :].astype(NDT))
    # fold LN2 affine into W1; swizzle to [hc][p][ec*128]
    W1 = np.asarray(inputs["W1"], f32)
    w1_f = (g2[:, None] * W1).astype(NDT)                    # [1024, 4096]
    w1_sw = np.ascontiguousarray(
        w1_f.reshape(EC, 128, HC, 128).transpose(2, 1, 0, 3)
        .reshape(HC, 128, EC * 128))
    c1_f = np.asarray(inputs["c1"], f32) + beta2 @ W1
    c1_t = np.ascontiguousarray(c1_f.reshape(HC, 128).T)     # [128, HC]
    # W2 swizzle to [half][ec][p][j*128]
    W2 = np.asarray(inputs["W2"], f32).astype(NDT)           # [4096, 1024]
    w2_sw = np.ascontiguousarray(
        W2.reshape(2, HC // 2, 128, EC, 128).transpose(0, 3, 2, 1, 4)
        .reshape(2, EC, 128, (HC // 2) * 128))
    c2 = np.ascontiguousarray(np.asarray(inputs["c2"], f32)[None, :].astype(NDT))
    Wfc = np.asarray(inputs["Wfc"], f32)
    bfc = np.asarray(inputs["bfc"], f32)

    NVC = VOCAB // VCW
    wfc_sw = np.ascontiguousarray(
        Wfc.astype(NDT).reshape(EC, 128, NVC, VCW).transpose(2, 1, 0, 3)
        .reshape(NVC, 128, EC * VCW))
    bfc_sw = np.ascontiguousarray(bfc.astype(NDT).reshape(NVC, VCW))
    in_maps = []
    for c in range(N_CORES):
        mb = np.zeros((128, GRP), np.float32)
        mb[:, c % GRP] = -1e4
        in_maps.append(dict(
            emb=emb,
            xi=np.ascontiguousarray(x[c * T:(c + 1) * T, None]),
            wq=wq_f, wk=wk_f, wv=wv_f, bqkv=bqkv,
            wd=wd_h, bd=bd, w1=w1_sw, c1=c1_t, w2=w2_sw, c2=c2,
            wfc=wfc_sw, bfc=bfc_sw, mbias=mb,
        ))
    return in_maps


def kernel(**inputs) -> np.ndarray:
    nc = _get_nc()
    in_maps = prepare_in_maps(inputs)
    r = run_bass_kernel_spmd(nc, in_maps, core_ids=list(range(N_CORES)))
    logits = np.concatenate([r.results[c]["out"] for c in range(N_CORES)], axis=0)
    return logits.reshape(BATCH, SEQ, VOCAB)
